# revision 49
# baseline (speedup 1.0000x reference)
"""GCNConv (N=100000 nodes, d=64, E=1.6M edges) on 8 Trainium2 NeuronCores.

Formula (DGL GraphConv, in==out feats):
    out_deg = bincount(src); in_deg = bincount(dst)
    norm_src = clip(out_deg,1)^-0.5 ; norm_dst = clip(in_deg,1)^-0.5
    feat = x * norm_src[:,None]
    agg[d] = sum_{e: dst[e]=d} feat[src[e]]
    out = (agg * norm_dst[:,None]) @ W

Distribution: nodes sharded 8 ways (12500/core).
  Phase 1 (core k, edges with src in shard k): out-degree histogram over
    32-node windows (DVE one-hot + free-axis reduce + tiny count matmul into
    a PSUM degree row per 128-node block); per block: PE-transpose the raw
    degree row to a column, clip/sqrt(ACT)/recip on [128,1], ACT row-scale
    the x block, write feat shard [12500, 128] bf16 (col 64 = 1.0 ->
    in-degree for free in phase 2; cols 65..127 zero pad to a 256B row for
    the SWDGE gather).
  AllGather in 4 pieces (one per block-aligned shard slice; piece p doubles
    as int16 gather segment p of <=25600 rows). Each piece is issued from a
    tile_critical as soon as its blocks are written, so collectives overlap
    the remainder of phase 1; phase-2 gathers gate on ccsem >= p+1.
  Phase 2 (core k, edges with dst in shard k): edges bucketed by
    (128-node dst window, segment); superchunks of GW=4 windows. Tiles of
    128 edges are gathered in batches of up to 8 tiles (1024 rows) with ONE
    gpsimd.dma_gather per batch, round-robined over 4 SWDGE queues (the
    994ns fixed SWDGE cost is amortized 8x and the 1024-descriptor ring
    drains overlap across queues; a single queue serializes). Per tile: a
    one-hot scatter matmul accumulates into a per-window single-bank PSUM
    tile [65, 128] (row 64 = in_deg). Windows accumulate strictly
    window-major: interleaving accumulation regions within a PSUM bank
    corrupts results (start appears to arm per bank, not per region).
    Per window: norm_dst via deg-row transpose -> [128,1] clip/sqrt/recip,
    agg copied to SBUF bf16 on ACT, out_blk = agg^T @ W, ACT row-scale,
    store.

Host side only shards/buckets edges and builds index/window inputs; all
arithmetic of the formula (degrees, norms, scaling, aggregation, matmul)
runs on device.

Perf journey (HW exec): 3084us baseline (per-tile indirect DMA, gpsimd
desc-gen bound) -> 1284us (batched dma_gather, 4 queues) -> 1234us
(pieced AllGather overlap) -> 1146us (ACT offload of row-scales/copies).
A 2-pass segment-split (start gathers before phase 1 ends) was tried and
reverted: halved per-superchunk buffers shallow the gather pipeline and
cost more than the earlier start gained.
"""

import sys

if "/opt/trn_rl_repo" not in sys.path:
    sys.path.insert(0, "/opt/trn_rl_repo")

import numpy as np

import concourse.bass as bass
import concourse.mybir as mybir
import concourse.tile as tile
from concourse.library_config import mlp as _mlp_lib

N_NODES = 100000
D = 64
N_CORES = 8
SHARD = N_NODES // N_CORES  # 12500
W1 = 32  # phase-1 (degree-count) window width
W2 = 128  # phase-2 dst window == node block
P = 128  # edges per tile (matmul contraction dim)
CHUNK1 = 64  # phase-1 max tiles per chunk (window-aligned packing)
ELEM = 128  # gather row width in bf16 (256 B)
NSEG = 4  # int16 gather table segments == AllGather pieces
PSTART = [0, 3200, 6400, 9600]  # piece starts within a shard (block-aligned)
PSZ = [3200, 3200, 3200, 2900]  # piece sizes; table_p = 8*PSZ[p] rows < 2**15
GW = 8  # dst windows per superchunk (per-window single-bank PSUM tiles)
MAXG = 8  # tiles per dma_gather (8*128 = 1024 rows; HW ring caps ~1024)
NSWQ = 4  # SWDGE queues; gather calls round-robin across them

F32 = mybir.dt.float32
BF16 = mybir.dt.bfloat16
I16 = mybir.dt.int16

MD = BF16


def split_waits(nc, maxw=1):
    """This walrus build allows at most `maxw` sem-waits per instruction;
    move extras onto preceding InstEventSemaphore carriers (same engine)."""
    for f in nc.m.functions:
        for blk in f.blocks:
            newl = []
            for ins in blk.instructions:
                si = ins.sync_info
                if si is not None and si.on_wait and len(si.on_wait) > maxw:
                    waits = list(si.on_wait)
                    carry, keep = waits[:-maxw], waits[-maxw:]
                    for i in range(0, len(carry), maxw):
                        w = mybir.InstEventSemaphore(
                            name=nc.get_next_instruction_name(), ins=[], outs=[]
                        )
                        w.engine = ins.engine
                        w.sync_info = mybir.SyncInfo(
                            on_wait=carry[i : i + maxw], on_update=[]
                        )
                        newl.append(w)
                    ins.sync_info = mybir.SyncInfo(
                        on_wait=keep, on_update=list(si.on_update)
                    )
                newl.append(ins)
            blk.instructions[:] = newl


def hoist_library_reload(nc):
    """Move the gpsimd library-reload pseudo inst ahead of the first Pool
    instruction so the mlp ucode (dma_gather) is resident before use."""
    import concourse.bass_isa as bass_isa

    for f in nc.m.functions:
        for blk in f.blocks:
            insts = blk.instructions
            ri = next(
                (
                    i
                    for i, ins in enumerate(insts)
                    if isinstance(ins, bass_isa.InstPseudoReloadLibraryIndex)
                ),
                None,
            )
            if ri is None:
                continue
            pi = next(
                (
                    i
                    for i, ins in enumerate(insts)
                    if ins.engine == mybir.EngineType.Pool
                    and not isinstance(ins, bass_isa.InstPseudoReloadLibraryIndex)
                ),
                None,
            )
            if pi is not None and pi < ri:
                reload = insts.pop(ri)
                insts.insert(pi, reload)


def _layout(cnts_per_core):
    """Uniform (max-over-cores) tiles per window."""
    tiles_w = (cnts_per_core.max(axis=0) + P - 1) // P
    tbase = np.concatenate([[0], np.cumsum(tiles_w)[:-1]])
    return tiles_w.astype(np.int64), tbase.astype(np.int64), int(tiles_w.sum())


def _prep(x, W, src, dst):
    """Host-side sharding: bucket edges by shard/window/segment, build
    per-core device inputs and the shared (uniform) tile metadata."""
    import ml_dtypes

    src = np.asarray(src)
    dst = np.asarray(dst)
    x = np.asarray(x, dtype=np.float32)
    W = np.asarray(W, dtype=np.float32)

    nwin1 = (SHARD + W1 - 1) // W1
    nwin2 = (SHARD + W2 - 1) // W2

    per_core = []
    c1 = np.zeros((N_CORES, nwin1), dtype=np.int64)
    c2 = np.zeros((N_CORES, nwin2 * NSEG), dtype=np.int64)
    for k in range(N_CORES):
        sel1 = (src // SHARD) == k
        loc1 = src[sel1] - SHARD * k
        w1v = loc1 // W1
        c1[k] = np.bincount(w1v, minlength=nwin1)

        sel2 = (dst // SHARD) == k
        loc2 = dst[sel2] - SHARD * k
        gidx = src[sel2].astype(np.int64)
        wv = loc2 // W2
        slot = (loc2 % W2).astype(np.float32)
        gs = gidx // SHARD  # owning shard of the src node
        off = gidx - gs * SHARD
        qv = np.minimum(off // 3200, NSEG - 1)  # AllGather piece == segment
        lidx = gs * np.asarray(PSZ)[qv] + (off - np.asarray(PSTART)[qv])
        key = wv * NSEG + qv
        c2[k] = np.bincount(key, minlength=nwin2 * NSEG)
        per_core.append((loc1, w1v, key, slot, lidx))

    t1_w, t1_base, T1 = _layout(c1)

    # ---- phase-2 layout: superchunks of GW windows, quarter-major inside ----
    t2_wq = ((c2.max(axis=0) + P - 1) // P).astype(np.int64)  # [nwin2*NSEG]
    tile_base = np.zeros(nwin2 * NSEG, dtype=np.int64)
    sc_list = []
    win_first = {}
    win_last = {}
    t = 0
    for w0 in range(0, nwin2, GW):
        ws = list(range(w0, min(w0 + GW, nwin2)))
        sc = {"w0": w0, "ws": ws, "t0": t, "wtiles": {w: [] for w in ws}, "calls": []}
        for q in range(NSEG):
            run_t0 = t
            for w in ws:
                keyi = w * NSEG + q
                n = int(t2_wq[keyi])
                if n == 0:
                    continue
                tile_base[keyi] = t
                for _ in range(n):
                    if w not in win_first:
                        win_first[w] = t
                    win_last[w] = t
                    sc["wtiles"][w].append(t - sc["t0"])
                    t += 1
            nrun = t - run_t0
            o = 0
            while o < nrun:
                n = min(MAXG, nrun - o)
                sc["calls"].append((q, run_t0 - sc["t0"] + o, n))
                o += n
        sc["nt"] = t - sc["t0"]
        # pass A = segments 0-1 (a prefix of the sc's tiles), pass B = 2-3
        sc["nA"] = sum(
            int(t2_wq[w * NSEG + q]) for q in range(NSEG // 2) for w in ws
        )
        sc["wtiles_A"] = {w: [lt for lt in sc["wtiles"][w] if lt < sc["nA"]] for w in ws}
        sc["wtiles_B"] = {w: [lt for lt in sc["wtiles"][w] if lt >= sc["nA"]] for w in ws}
        sc["calls_A"] = [c for c in sc["calls"] if c[0] < NSEG // 2]
        sc["calls_B"] = [c for c in sc["calls"] if c[0] >= NSEG // 2]
        sc_list.append(sc)
    T2 = t

    bf16 = ml_dtypes.bfloat16
    iota1 = np.broadcast_to(np.arange(W1, dtype=np.float32), (P, W1)).astype(bf16)
    iota2 = np.broadcast_to(np.arange(W2, dtype=np.float32), (P, W2)).astype(bf16)
    ones = np.ones((P, 1), dtype=np.float32)
    ones_m = np.ones((P, 1), dtype=bf16)
    ident = np.eye(D + 1, dtype=np.float32).astype(bf16)
    w64 = W.astype(bf16)

    ins_maps = []
    for k in range(N_CORES):
        loc1, w1v, key, slot, lidx = per_core[k]

        # phase-1 window map (as before)
        order1 = np.argsort(w1v, kind="stable")
        ws1 = w1v[order1]
        cnt1 = np.bincount(w1v, minlength=nwin1)
        starts1 = np.concatenate([[0], np.cumsum(cnt1)[:-1]])
        rank1 = np.arange(len(order1)) - starts1[ws1]
        col1 = t1_base[ws1] + rank1 // P
        lane1 = rank1 % P
        p1win = np.full((P, T1), float(W1), dtype=np.float32)
        p1win[lane1, col1] = (loc1[order1] - W1 * ws1).astype(np.float32)
        p1win = p1win.astype(bf16)

        # phase-2: slot codes + wrapped int16 gather indices
        order = np.argsort(key, kind="stable")
        ks = key[order]
        cnt = np.bincount(key, minlength=nwin2 * NSEG)
        starts = np.concatenate([[0], np.cumsum(cnt)[:-1]])
        rank = np.arange(len(order)) - starts[ks]
        tau = tile_base[ks] + rank // P
        lane = rank % P
        p2win = np.full((P, T2), float(W2), dtype=np.float32)
        p2win[lane, tau] = slot[order]
        p2win = p2win.astype(bf16)
        idx16 = np.zeros((16, T2 * 8), dtype=np.int16)
        idx16[lane % 16, tau * 8 + lane // 16] = lidx[order].astype(np.int16)
        p2idx = np.tile(idx16, (8, 1))

        ins_maps.append(
            {
                "xs": np.ascontiguousarray(x[SHARD * k : SHARD * (k + 1)]),
                "p1win": p1win,
                "p2win": p2win,
                "p2idx": p2idx,
                "w64": w64,
                "iota1": iota1,
                "iota2": iota2,
                "ones": ones,
                "ones_m": ones_m,
                "ident": ident,
            }
        )

    meta = {
        "T1": T1,
        "T2": T2,
        "t1_w": t1_w,
        "nwin1": nwin1,
        "nwin2": nwin2,
        "sc_list": sc_list,
        "win_first": win_first,
        "win_last": win_last,
    }
    return ins_maps, meta


def _tile_maps(meta):
    # phase-1: pack whole windows into chunks of <= CHUNK1 tiles.
    chunks1 = []
    cur = []
    t0 = 0
    pos = 0
    for w, n in enumerate(meta["t1_w"]):
        n = int(n)
        if n == 0:
            continue
        if pos + n > CHUNK1 and cur:
            chunks1.append((t0, pos, cur))
            t0 += pos
            pos = 0
            cur = []
        cur.append((w, pos, pos + n))
        pos += n
    if cur:
        chunks1.append((t0, pos, cur))
    meta["p1_chunks"] = chunks1
    last_win_of_blk = {}
    for w, n in enumerate(meta["t1_w"]):
        if int(n) > 0:
            last_win_of_blk[w // 4] = w
    meta["p1_last_win_of_blk"] = last_win_of_blk
    return meta


def _build_nc(meta, do_split_waits=True, dbg=False):
    T1, T2 = meta["T1"], meta["T2"]
    t1_w = meta["t1_w"]
    nwin2 = meta["nwin2"]
    sc_list = meta["sc_list"]
    win_first = meta["win_first"]
    win_last = meta["win_last"]
    nt_max = max(sc["nt"] for sc in sc_list)

    nc = bass.Bass(num_swdge_queues=NSWQ)
    xs = nc.declare_dram_parameter("xs", [SHARD, D], F32, isOutput=False)
    p1win_d = nc.declare_dram_parameter("p1win", [P, T1], MD, isOutput=False)
    p2win_d = nc.declare_dram_parameter("p2win", [P, T2], MD, isOutput=False)
    p2idx_d = nc.declare_dram_parameter("p2idx", [P, T2 * 8], I16, isOutput=False)
    w64_d = nc.declare_dram_parameter("w64", [D, D], MD, isOutput=False)
    iota1_d = nc.declare_dram_parameter("iota1", [P, W1], MD, isOutput=False)
    iota2_d = nc.declare_dram_parameter("iota2", [P, W2], MD, isOutput=False)
    ones_d = nc.declare_dram_parameter("ones", [P, 1], F32, isOutput=False)
    onesm_d = nc.declare_dram_parameter("ones_m", [P, 1], MD, isOutput=False)
    ident_d = nc.declare_dram_parameter("ident", [D + 1, D + 1], MD, isOutput=False)
    out_d = nc.declare_dram_parameter("out", [SHARD, D], F32, isOutput=True)

    feat_s = nc.dram_tensor("feat_s", [SHARD, ELEM], MD)
    feat_fp = [
        nc.dram_tensor(f"feat_f{p}", [N_CORES * PSZ[p], ELEM], MD)
        for p in range(NSEG)
    ]
    if dbg:
        nt0 = sc_list[0]["nt"]
        dbg_gd_d = nc.declare_dram_parameter("dbg_gd", [P, nt0, ELEM], MD, isOutput=True)
        dbg_oh_d = nc.declare_dram_parameter("dbg_oh", [P, nt0, W2], MD, isOutput=True)

    with tile.TileContext(nc) as tc:
        with tc.tile_pool(name="consts", bufs=1) as consts:
            nc.gpsimd.load_library(_mlp_lib)
            w64_sb = consts.tile([D, D], MD, tag="w64")
            iota1_sb = consts.tile([P, W1], MD, tag="iota1")
            iota2_sb = consts.tile([P, W2], MD, tag="iota2")
            ones_sb = consts.tile([P, 1], F32, tag="ones")
            onesm_sb = consts.tile([P, 1], MD, tag="onesm")
            ident_sb = consts.tile([D + 1, D + 1], MD, tag="ident")
            nc.sync.dma_start(out=w64_sb[:], in_=w64_d[:])
            nc.sync.dma_start(out=iota1_sb[:], in_=iota1_d[:])
            nc.sync.dma_start(out=iota2_sb[:], in_=iota2_d[:])
            nc.sync.dma_start(out=ones_sb[:], in_=ones_d[:])
            nc.sync.dma_start(out=onesm_sb[:], in_=onesm_d[:])
            nc.sync.dma_start(out=ident_sb[:], in_=ident_d[:])
            ccsem = nc.alloc_semaphore("ccsem")

            # ---------------- phase 1: out-degree -> feat shard -------------
            with (
                tc.tile_pool(name="p1win", bufs=2) as p_win,
                tc.tile_pool(name="p1oh", bufs=2) as p_oh,
                tc.tile_pool(name="p1s", bufs=4) as p_s,
                tc.tile_pool(name="p1ps", bufs=2, space="PSUM") as p_ps,
                tc.tile_pool(name="p1trps", bufs=2, space="PSUM") as p_trps,
                tc.tile_pool(name="p1x", bufs=2) as p_x,
                tc.tile_pool(name="p1feat", bufs=2) as p_feat,
                tc.tile_pool(name="p1misc", bufs=4) as p_misc,
            ):
                ps_blk = {}

                def p1_block_epilogue(b, ps):
                    for j2 in range(4):
                        w2 = 4 * b + j2
                        if w2 >= meta["nwin1"] or t1_w[w2] == 0:
                            nc.vector.memset(ps[:, W1 * j2 : W1 * (j2 + 1)], 0.0)
                    # raw degree row [1,128] -> SBUF -> PE transpose -> [128,1]
                    rowc = p_misc.tile([1, P], F32, tag="m_row")
                    nc.vector.tensor_copy(rowc[:], ps[:])
                    tp = p_trps.tile([P, 1], F32)
                    nc.tensor.matmul(
                        out=tp[:],
                        lhsT=rowc[:],
                        rhs=ones_sb[0:1, 0:1],
                        start=True,
                        stop=True,
                    )
                    dcl = p_misc.tile([P, 1], F32, tag="m_dcl")
                    nc.vector.tensor_scalar_max(dcl[:], tp[:], 1.0)
                    dsq = p_misc.tile([P, 1], F32, tag="m_dsq")
                    nc.scalar.sqrt(dsq[:], dcl[:])
                    ncol = p_misc.tile([P, 1], F32, tag="m_ncol")
                    nc.vector.reciprocal(ncol[:], dsq[:])
                    nb = min(P, SHARD - P * b)
                    xb = p_x.tile([P, D], F32, tag="xb")
                    nc.sync.dma_start(out=xb[:nb], in_=xs[P * b : P * b + nb, :])
                    fb = p_feat.tile([P, ELEM], MD, tag="fb")
                    nc.scalar.mul(fb[:, 0:D], xb[:], ncol[:])
                    nc.vector.memset(fb[:, D : D + 1], 1.0)
                    nc.vector.memset(fb[:, D + 1 : ELEM], 0.0)
                    nc.sync.dma_start(
                        out=feat_s[P * b : P * b + nb, :], in_=fb[:nb, :]
                    )

                # AllGather piece p covers shard rows [PSTART[p], PSTART[p]+PSZ[p]);
                # issued as soon as its last 128-row block is written, overlapping
                # the collective with the rest of phase 1.
                piece_end_blk = {
                    (PSTART[p] + PSZ[p] + P - 1) // P - 1: p for p in range(NSEG)
                }

                def emit_allgather(p):
                    with tc.tile_critical():
                        nc.gpsimd.collective_compute(
                            "AllGather",
                            mybir.AluOpType.bypass,
                            replica_groups=[list(range(N_CORES))],
                            ins=[feat_s[PSTART[p] : PSTART[p] + PSZ[p], :]],
                            outs=[feat_fp[p][:]],
                        ).then_inc(ccsem, 1)

                def maybe_allgather(b):
                    p = piece_end_blk.get(b)
                    if p is not None:
                        emit_allgather(p)

                for t0, cw, wins in meta["p1_chunks"]:
                    wt = p_win.tile([P, CHUNK1], MD, tag="wt")
                    nc.sync.dma_start(out=wt[:, :cw], in_=p1win_d[:, t0 : t0 + cw])
                    oh = p_oh.tile([P, W1, CHUNK1], MD, tag="oh")
                    nc.vector.tensor_tensor(
                        out=oh[:, :, :cw],
                        in0=wt[:, None, :cw].to_broadcast([P, W1, cw]),
                        in1=iota1_sb[:, :, None].to_broadcast([P, W1, cw]),
                        op=mybir.AluOpType.is_equal,
                    )
                    for w, a, bnd in wins:
                        S = p_s.tile([P, W1, 1], MD, tag="S")
                        with nc.allow_low_precision(
                            reason="one-hot counts <=64 are exact in bf16"
                        ):
                            nc.vector.tensor_reduce(
                                out=S[:],
                                in_=oh[:, :, a:bnd],
                                axis=mybir.AxisListType.X,
                                op=mybir.AluOpType.add,
                            )
                        b, j = w // 4, w % 4
                        if b not in ps_blk:
                            ps_blk[b] = p_ps.tile([1, P], F32, name="psblk", tag="psblk")
                        nc.tensor.matmul(
                            out=ps_blk[b][:, W1 * j : W1 * (j + 1)],
                            lhsT=onesm_sb[:],
                            rhs=S[:, :, 0],
                            start=True,
                            stop=True,
                        )
                        if w == meta["p1_last_win_of_blk"].get(b, -1):
                            p1_block_epilogue(b, ps_blk.pop(b))
                            maybe_allgather(b)

            # -------- phase 2: batched gather + scatter matmul + W ----------
            with (
                tc.tile_pool(name="p2i", bufs=2) as p_idx,
                tc.tile_pool(name="p2w", bufs=2) as p_win2,
                tc.tile_pool(name="p2g", bufs=2) as p_g,
                tc.tile_pool(name="p2oh", bufs=2) as p_oh2,
                tc.tile_pool(name="p2ps", bufs=3, space="PSUM") as p_ps2,
                tc.tile_pool(name="p2tr", bufs=2, space="PSUM") as p_tr2,
                tc.tile_pool(name="p2ops", bufs=2, space="PSUM") as p_ops,
                tc.tile_pool(name="p2mrg", bufs=3) as p_mrg,
                tc.tile_pool(name="p2out", bufs=2) as p_out,
                tc.tile_pool(name="p2misc", bufs=4) as p_misc2,
            ):
                nreg = {}
                for sc in sc_list:
                    for q, lt0, n in sc["calls"]:
                        if n * P not in nreg:
                            nreg[n * P] = nc.gpsimd.to_reg(n * P)
                gcall_i = 0
                seg_waited = set()
                for sc in sc_list:
                    nt = sc["nt"]
                    if nt == 0:
                        continue
                    t0 = sc["t0"]
                    ix = p_idx.tile([P, nt_max * 8], I16, tag="ix")
                    nc.sync.dma_start(
                        out=ix[:, : nt * 8], in_=p2idx_d[:, t0 * 8 : (t0 + nt) * 8]
                    )
                    wt = p_win2.tile([P, nt_max], MD, tag="wt2")
                    nc.sync.dma_start(out=wt[:, :nt], in_=p2win_d[:, t0 : t0 + nt])
                    gd = p_g.tile([P, nt_max, ELEM], MD, tag="gd")
                    oh = p_oh2.tile([P, nt_max, W2], MD, tag="oh2")
                    for q, lt0, n in sc["calls"]:
                        if q not in seg_waited:
                            with tc.tile_critical():
                                nc.gpsimd.wait_ge(ccsem, q + 1)
                            seg_waited.add(q)
                        nc.gpsimd.dma_gather(
                            gd[:, lt0 : lt0 + n, :],
                            feat_fp[q][:],
                            ix[:, lt0 * 8 : (lt0 + n) * 8],
                            n * P,
                            nreg[n * P],
                            ELEM,
                            queue_num=gcall_i % NSWQ,
                        )
                        gcall_i += 1
                        nc.vector.tensor_tensor(
                            out=oh[:, lt0 : lt0 + n, :],
                            in0=wt[:, lt0 : lt0 + n, None].to_broadcast([P, n, W2]),
                            in1=iota2_sb[:, None, :].to_broadcast([P, n, W2]),
                            op=mybir.AluOpType.is_equal,
                        )
                    for w in sc["ws"]:
                        lts = sc["wtiles"][w]
                        if not lts:
                            continue
                        ps = p_ps2.tile([D + 1, W2], F32, tag="ps2")
                        for i, lt in enumerate(lts):
                            nc.tensor.matmul(
                                out=ps[:],
                                lhsT=gd[:, lt, 0 : D + 1],
                                rhs=oh[:, lt, :],
                                start=(i == 0),
                                stop=(i == len(lts) - 1),
                            )
                        # norm_dst from the exact deg row, via transpose
                        rowc = p_misc2.tile([1, P], F32, tag="d_row")
                        nc.vector.tensor_copy(rowc[:], ps[D : D + 1, :])
                        tp2 = p_tr2.tile([P, 1], F32)
                        nc.tensor.matmul(
                            out=tp2[:],
                            lhsT=rowc[:],
                            rhs=ones_sb[0:1, 0:1],
                            start=True,
                            stop=True,
                        )
                        dcl = p_misc2.tile([P, 1], F32, tag="d_dcl")
                        nc.vector.tensor_scalar_max(dcl[:], tp2[:], 1.0)
                        dsq = p_misc2.tile([P, 1], F32, tag="d_dsq")
                        nc.scalar.sqrt(dsq[:], dcl[:])
                        drr = p_misc2.tile([P, 1], F32, tag="d_drr")
                        nc.vector.reciprocal(drr[:], dsq[:])
                        ag = p_mrg.tile([D, P], MD, tag="agf")
                        nc.scalar.copy(ag[:], ps[0:D, :])
                        op = p_ops.tile([P, D], F32)
                        nc.tensor.matmul(
                            out=op[:],
                            lhsT=ag[:],
                            rhs=w64_sb[:],
                            start=True,
                            stop=True,
                        )
                        ob = p_out.tile([P, D], F32, tag="ob")
                        nc.scalar.mul(ob[:], op[:], drr[:])
                        nb = min(P, SHARD - W2 * w)
                        nc.sync.dma_start(
                            out=out_d[W2 * w : W2 * w + nb, :], in_=ob[:nb, :]
                        )
                # windows with no edges anywhere: write zeros
                for w in range(nwin2):
                    if w not in win_first:
                        zb = p_out.tile([P, D], F32, tag="ob")
                        nc.vector.memset(zb[:], 0.0)
                        nb = min(P, SHARD - W2 * w)
                        nc.sync.dma_start(
                            out=out_d[W2 * w : W2 * w + nb, :], in_=zb[:nb, :]
                        )

    if do_split_waits:
        split_waits(nc)
    hoist_library_reload(nc)
    mybir.codegen_inst_isa_subclasses(nc)
    return nc


def kernel(x, W, src, dst):
    from concourse.bass_utils import run_bass_kernel_spmd

    ins_maps, meta = _prep(x, W, src, dst)
    meta = _tile_maps(meta)
    nc = _build_nc(meta)
    res = run_bass_kernel_spmd(nc, ins_maps, list(range(N_CORES)))
    out = np.concatenate([res.results[k]["out"] for k in range(N_CORES)], axis=0)
    return out.astype(np.float32)


# revision 50
# speedup vs baseline: 1.2098x; 1.2098x over previous
"""GCNConv (N=100000 nodes, d=64, E=1.6M edges) on 8 Trainium2 NeuronCores.

Formula (DGL GraphConv, in==out feats):
    out_deg = bincount(src); in_deg = bincount(dst)
    norm_src = clip(out_deg,1)^-0.5 ; norm_dst = clip(in_deg,1)^-0.5
    feat = x * norm_src[:,None]
    agg[d] = sum_{e: dst[e]=d} feat[src[e]]
    out = (agg * norm_dst[:,None]) @ W

Distribution: nodes sharded 8 ways (12500/core).
  Phase 1 (core k, edges with src in shard k): out-degree histogram over
    32-node windows (DVE one-hot + free-axis reduce + tiny count matmul into
    a PSUM degree row per 128-node block); per block: PE-transpose the raw
    degree row to a column, clip/sqrt(ACT)/recip on [128,1], ACT row-scale
    the x block, write feat shard [12500, 128] bf16 (col 64 = 1.0 ->
    in-degree for free in phase 2; cols 65..127 zero pad to a 256B row for
    the SWDGE gather).
  AllGather in 4 pieces (one per block-aligned shard slice; piece p doubles
    as int16 gather segment p of <=25600 rows). Each piece is issued from a
    tile_critical as soon as its blocks are written, so collectives overlap
    the remainder of phase 1; phase-2 gathers gate on ccsem >= p+1.
  Phase 2 (core k, edges with dst in shard k): edges bucketed by
    (128-node dst window, segment); superchunks of GW=4 windows. Tiles of
    128 edges are gathered in batches of up to 8 tiles (1024 rows) with ONE
    gpsimd.dma_gather per batch, round-robined over 4 SWDGE queues (the
    994ns fixed SWDGE cost is amortized 8x and the 1024-descriptor ring
    drains overlap across queues; a single queue serializes). Per tile: a
    one-hot scatter matmul accumulates into a per-window single-bank PSUM
    tile [65, 128] (row 64 = in_deg). Windows accumulate strictly
    window-major: interleaving accumulation regions within a PSUM bank
    corrupts results (start appears to arm per bank, not per region).
    Per window: norm_dst via deg-row transpose -> [128,1] clip/sqrt/recip,
    agg copied to SBUF bf16 on ACT, out_blk = agg^T @ W, ACT row-scale,
    store.

Host side only shards/buckets edges and builds index/window inputs; all
arithmetic of the formula (degrees, norms, scaling, aggregation, matmul)
runs on device.

Perf journey (HW exec): 3084us baseline (per-tile indirect DMA, gpsimd
desc-gen bound) -> 1284us (batched dma_gather, 4 queues) -> 1234us
(pieced AllGather overlap) -> 1146us (ACT offload of row-scales/copies).
A 2-pass segment-split (start gathers before phase 1 ends) was tried and
reverted: halved per-superchunk buffers shallow the gather pipeline and
cost more than the earlier start gained.
"""

import sys

if "/opt/trn_rl_repo" not in sys.path:
    sys.path.insert(0, "/opt/trn_rl_repo")

import numpy as np

import concourse.bass as bass
import concourse.mybir as mybir
import concourse.tile as tile
from concourse.library_config import mlp as _mlp_lib

N_NODES = 100000
D = 64
N_CORES = 8
SHARD = N_NODES // N_CORES  # 12500
W1 = 32  # phase-1 (degree-count) window width
W2 = 128  # phase-2 dst window == node block
P = 128  # edges per tile (matmul contraction dim)
CHUNK1 = 64  # phase-1 max tiles per chunk (window-aligned packing)
ELEM = 128  # gather row width in bf16 (256 B)
NSEG = 4  # int16 gather table segments == AllGather pieces
PSTART = [0, 3200, 6400, 9600]  # piece starts within a shard (block-aligned)
PSZ = [3200, 3200, 3200, 2900]  # piece sizes; table_p = 8*PSZ[p] rows < 2**15
GW = 4  # dst windows per superchunk (PSUM block [65, GW*128] = 1 bank)
MAXG = 8  # tiles per dma_gather (8*128 = 1024 rows; HW ring caps ~1024)
NSWQ = 4  # SWDGE queues; gather calls round-robin across them

F32 = mybir.dt.float32
BF16 = mybir.dt.bfloat16
I16 = mybir.dt.int16

MD = BF16


def split_waits(nc, maxw=1):
    """This walrus build allows at most `maxw` sem-waits per instruction;
    move extras onto preceding InstEventSemaphore carriers (same engine)."""
    for f in nc.m.functions:
        for blk in f.blocks:
            newl = []
            for ins in blk.instructions:
                si = ins.sync_info
                if si is not None and si.on_wait and len(si.on_wait) > maxw:
                    waits = list(si.on_wait)
                    carry, keep = waits[:-maxw], waits[-maxw:]
                    for i in range(0, len(carry), maxw):
                        w = mybir.InstEventSemaphore(
                            name=nc.get_next_instruction_name(), ins=[], outs=[]
                        )
                        w.engine = ins.engine
                        w.sync_info = mybir.SyncInfo(
                            on_wait=carry[i : i + maxw], on_update=[]
                        )
                        newl.append(w)
                    ins.sync_info = mybir.SyncInfo(
                        on_wait=keep, on_update=list(si.on_update)
                    )
                newl.append(ins)
            blk.instructions[:] = newl


def hoist_library_reload(nc):
    """Move the gpsimd library-reload pseudo inst ahead of the first Pool
    instruction so the mlp ucode (dma_gather) is resident before use."""
    import concourse.bass_isa as bass_isa

    for f in nc.m.functions:
        for blk in f.blocks:
            insts = blk.instructions
            ri = next(
                (
                    i
                    for i, ins in enumerate(insts)
                    if isinstance(ins, bass_isa.InstPseudoReloadLibraryIndex)
                ),
                None,
            )
            if ri is None:
                continue
            pi = next(
                (
                    i
                    for i, ins in enumerate(insts)
                    if ins.engine == mybir.EngineType.Pool
                    and not isinstance(ins, bass_isa.InstPseudoReloadLibraryIndex)
                ),
                None,
            )
            if pi is not None and pi < ri:
                reload = insts.pop(ri)
                insts.insert(pi, reload)


def _layout(cnts_per_core):
    """Uniform (max-over-cores) tiles per window."""
    tiles_w = (cnts_per_core.max(axis=0) + P - 1) // P
    tbase = np.concatenate([[0], np.cumsum(tiles_w)[:-1]])
    return tiles_w.astype(np.int64), tbase.astype(np.int64), int(tiles_w.sum())


def _prep(x, W, src, dst):
    """Host-side sharding: bucket edges by shard/window/segment, build
    per-core device inputs and the shared (uniform) tile metadata."""
    import ml_dtypes

    src = np.asarray(src)
    dst = np.asarray(dst)
    x = np.asarray(x, dtype=np.float32)
    W = np.asarray(W, dtype=np.float32)

    nwin1 = (SHARD + W1 - 1) // W1
    nwin2 = (SHARD + W2 - 1) // W2

    per_core = []
    c1 = np.zeros((N_CORES, nwin1), dtype=np.int64)
    c2 = np.zeros((N_CORES, nwin2 * NSEG), dtype=np.int64)
    for k in range(N_CORES):
        sel1 = (src // SHARD) == k
        loc1 = src[sel1] - SHARD * k
        w1v = loc1 // W1
        c1[k] = np.bincount(w1v, minlength=nwin1)

        sel2 = (dst // SHARD) == k
        loc2 = dst[sel2] - SHARD * k
        gidx = src[sel2].astype(np.int64)
        wv = loc2 // W2
        slot = (loc2 % W2).astype(np.float32)
        gs = gidx // SHARD  # owning shard of the src node
        off = gidx - gs * SHARD
        qv = np.minimum(off // 3200, NSEG - 1)  # AllGather piece == segment
        lidx = gs * np.asarray(PSZ)[qv] + (off - np.asarray(PSTART)[qv])
        key = wv * NSEG + qv
        c2[k] = np.bincount(key, minlength=nwin2 * NSEG)
        per_core.append((loc1, w1v, key, slot, lidx))

    t1_w, t1_base, T1 = _layout(c1)

    # ---- phase-2 layout: superchunks of GW windows, quarter-major inside ----
    t2_wq = ((c2.max(axis=0) + P - 1) // P).astype(np.int64)  # [nwin2*NSEG]
    tile_base = np.zeros(nwin2 * NSEG, dtype=np.int64)
    sc_list = []
    win_first = {}
    win_last = {}
    t = 0
    for w0 in range(0, nwin2, GW):
        ws = list(range(w0, min(w0 + GW, nwin2)))
        sc = {"w0": w0, "ws": ws, "t0": t, "wtiles": {w: [] for w in ws}, "calls": []}
        for q in range(NSEG):
            run_t0 = t
            for w in ws:
                keyi = w * NSEG + q
                n = int(t2_wq[keyi])
                if n == 0:
                    continue
                tile_base[keyi] = t
                for _ in range(n):
                    if w not in win_first:
                        win_first[w] = t
                    win_last[w] = t
                    sc["wtiles"][w].append(t - sc["t0"])
                    t += 1
            nrun = t - run_t0
            o = 0
            while o < nrun:
                n = min(MAXG, nrun - o)
                sc["calls"].append((q, run_t0 - sc["t0"] + o, n))
                o += n
        sc["nt"] = t - sc["t0"]
        # pass A = segments 0-1 (a prefix of the sc's tiles), pass B = 2-3
        sc["nA"] = sum(
            int(t2_wq[w * NSEG + q]) for q in range(NSEG // 2) for w in ws
        )
        sc["wtiles_A"] = {w: [lt for lt in sc["wtiles"][w] if lt < sc["nA"]] for w in ws}
        sc["wtiles_B"] = {w: [lt for lt in sc["wtiles"][w] if lt >= sc["nA"]] for w in ws}
        sc["calls_A"] = [c for c in sc["calls"] if c[0] < NSEG // 2]
        sc["calls_B"] = [c for c in sc["calls"] if c[0] >= NSEG // 2]
        sc_list.append(sc)
    T2 = t

    bf16 = ml_dtypes.bfloat16
    iota1 = np.broadcast_to(np.arange(W1, dtype=np.float32), (P, W1)).astype(bf16)
    iota2 = np.broadcast_to(np.arange(W2, dtype=np.float32), (P, W2)).astype(bf16)
    ones = np.ones((P, 1), dtype=np.float32)
    ones_m = np.ones((P, 1), dtype=bf16)
    ident = np.eye(D + 1, dtype=np.float32).astype(bf16)
    w64 = W.astype(bf16)

    ins_maps = []
    for k in range(N_CORES):
        loc1, w1v, key, slot, lidx = per_core[k]

        # phase-1 window map (as before)
        order1 = np.argsort(w1v, kind="stable")
        ws1 = w1v[order1]
        cnt1 = np.bincount(w1v, minlength=nwin1)
        starts1 = np.concatenate([[0], np.cumsum(cnt1)[:-1]])
        rank1 = np.arange(len(order1)) - starts1[ws1]
        col1 = t1_base[ws1] + rank1 // P
        lane1 = rank1 % P
        p1win = np.full((P, T1), float(W1), dtype=np.float32)
        p1win[lane1, col1] = (loc1[order1] - W1 * ws1).astype(np.float32)
        p1win = p1win.astype(bf16)

        # phase-2: slot codes + wrapped int16 gather indices
        order = np.argsort(key, kind="stable")
        ks = key[order]
        cnt = np.bincount(key, minlength=nwin2 * NSEG)
        starts = np.concatenate([[0], np.cumsum(cnt)[:-1]])
        rank = np.arange(len(order)) - starts[ks]
        tau = tile_base[ks] + rank // P
        lane = rank % P
        p2win = np.full((P, T2), float(W2), dtype=np.float32)
        p2win[lane, tau] = slot[order]
        p2win = p2win.astype(bf16)
        idx16 = np.zeros((16, T2 * 8), dtype=np.int16)
        idx16[lane % 16, tau * 8 + lane // 16] = lidx[order].astype(np.int16)
        p2idx = np.tile(idx16, (8, 1))

        ins_maps.append(
            {
                "xs": np.ascontiguousarray(x[SHARD * k : SHARD * (k + 1)]),
                "p1win": p1win,
                "p2win": p2win,
                "p2idx": p2idx,
                "w64": w64,
                "iota1": iota1,
                "iota2": iota2,
                "ones": ones,
                "ones_m": ones_m,
                "ident": ident,
            }
        )

    meta = {
        "T1": T1,
        "T2": T2,
        "t1_w": t1_w,
        "nwin1": nwin1,
        "nwin2": nwin2,
        "sc_list": sc_list,
        "win_first": win_first,
        "win_last": win_last,
    }
    return ins_maps, meta


def _tile_maps(meta):
    # phase-1: pack whole windows into chunks of <= CHUNK1 tiles.
    chunks1 = []
    cur = []
    t0 = 0
    pos = 0
    for w, n in enumerate(meta["t1_w"]):
        n = int(n)
        if n == 0:
            continue
        if pos + n > CHUNK1 and cur:
            chunks1.append((t0, pos, cur))
            t0 += pos
            pos = 0
            cur = []
        cur.append((w, pos, pos + n))
        pos += n
    if cur:
        chunks1.append((t0, pos, cur))
    meta["p1_chunks"] = chunks1
    last_win_of_blk = {}
    for w, n in enumerate(meta["t1_w"]):
        if int(n) > 0:
            last_win_of_blk[w // 4] = w
    meta["p1_last_win_of_blk"] = last_win_of_blk
    return meta


def _build_nc(meta, do_split_waits=True, dbg=False):
    T1, T2 = meta["T1"], meta["T2"]
    t1_w = meta["t1_w"]
    nwin2 = meta["nwin2"]
    sc_list = meta["sc_list"]
    win_first = meta["win_first"]
    win_last = meta["win_last"]
    nt_max = max(sc["nt"] for sc in sc_list)

    nc = bass.Bass(num_swdge_queues=NSWQ)
    xs = nc.declare_dram_parameter("xs", [SHARD, D], F32, isOutput=False)
    p1win_d = nc.declare_dram_parameter("p1win", [P, T1], MD, isOutput=False)
    p2win_d = nc.declare_dram_parameter("p2win", [P, T2], MD, isOutput=False)
    p2idx_d = nc.declare_dram_parameter("p2idx", [P, T2 * 8], I16, isOutput=False)
    w64_d = nc.declare_dram_parameter("w64", [D, D], MD, isOutput=False)
    iota1_d = nc.declare_dram_parameter("iota1", [P, W1], MD, isOutput=False)
    iota2_d = nc.declare_dram_parameter("iota2", [P, W2], MD, isOutput=False)
    ones_d = nc.declare_dram_parameter("ones", [P, 1], F32, isOutput=False)
    onesm_d = nc.declare_dram_parameter("ones_m", [P, 1], MD, isOutput=False)
    ident_d = nc.declare_dram_parameter("ident", [D + 1, D + 1], MD, isOutput=False)
    out_d = nc.declare_dram_parameter("out", [SHARD, D], F32, isOutput=True)

    feat_s = nc.dram_tensor("feat_s", [SHARD, ELEM], MD)
    feat_fp = [
        nc.dram_tensor(f"feat_f{p}", [N_CORES * PSZ[p], ELEM], MD)
        for p in range(NSEG)
    ]
    if dbg:
        nt0 = sc_list[0]["nt"]
        dbg_gd_d = nc.declare_dram_parameter("dbg_gd", [P, nt0, ELEM], MD, isOutput=True)
        dbg_oh_d = nc.declare_dram_parameter("dbg_oh", [P, nt0, W2], MD, isOutput=True)

    with tile.TileContext(nc) as tc:
        with tc.tile_pool(name="consts", bufs=1) as consts:
            nc.gpsimd.load_library(_mlp_lib)
            w64_sb = consts.tile([D, D], MD, tag="w64")
            iota1_sb = consts.tile([P, W1], MD, tag="iota1")
            iota2_sb = consts.tile([P, W2], MD, tag="iota2")
            ones_sb = consts.tile([P, 1], F32, tag="ones")
            onesm_sb = consts.tile([P, 1], MD, tag="onesm")
            ident_sb = consts.tile([D + 1, D + 1], MD, tag="ident")
            nc.sync.dma_start(out=w64_sb[:], in_=w64_d[:])
            nc.sync.dma_start(out=iota1_sb[:], in_=iota1_d[:])
            nc.sync.dma_start(out=iota2_sb[:], in_=iota2_d[:])
            nc.sync.dma_start(out=ones_sb[:], in_=ones_d[:])
            nc.sync.dma_start(out=onesm_sb[:], in_=onesm_d[:])
            nc.sync.dma_start(out=ident_sb[:], in_=ident_d[:])
            ccsem = nc.alloc_semaphore("ccsem")

            # ---------------- phase 1: out-degree -> feat shard -------------
            with (
                tc.tile_pool(name="p1win", bufs=2) as p_win,
                tc.tile_pool(name="p1oh", bufs=2) as p_oh,
                tc.tile_pool(name="p1s", bufs=4) as p_s,
                tc.tile_pool(name="p1ps", bufs=2, space="PSUM") as p_ps,
                tc.tile_pool(name="p1trps", bufs=2, space="PSUM") as p_trps,
                tc.tile_pool(name="p1x", bufs=2) as p_x,
                tc.tile_pool(name="p1feat", bufs=2) as p_feat,
                tc.tile_pool(name="p1misc", bufs=4) as p_misc,
            ):
                ps_blk = {}

                def p1_block_epilogue(b, ps):
                    for j2 in range(4):
                        w2 = 4 * b + j2
                        if w2 >= meta["nwin1"] or t1_w[w2] == 0:
                            nc.vector.memset(ps[:, W1 * j2 : W1 * (j2 + 1)], 0.0)
                    # raw degree row [1,128] -> SBUF -> PE transpose -> [128,1]
                    rowc = p_misc.tile([1, P], F32, tag="m_row")
                    nc.vector.tensor_copy(rowc[:], ps[:])
                    tp = p_trps.tile([P, 1], F32)
                    nc.tensor.matmul(
                        out=tp[:],
                        lhsT=rowc[:],
                        rhs=ones_sb[0:1, 0:1],
                        start=True,
                        stop=True,
                    )
                    dcl = p_misc.tile([P, 1], F32, tag="m_dcl")
                    nc.vector.tensor_scalar_max(dcl[:], tp[:], 1.0)
                    dsq = p_misc.tile([P, 1], F32, tag="m_dsq")
                    nc.scalar.sqrt(dsq[:], dcl[:])
                    ncol = p_misc.tile([P, 1], F32, tag="m_ncol")
                    nc.vector.reciprocal(ncol[:], dsq[:])
                    nb = min(P, SHARD - P * b)
                    xb = p_x.tile([P, D], F32, tag="xb")
                    nc.sync.dma_start(out=xb[:nb], in_=xs[P * b : P * b + nb, :])
                    fb = p_feat.tile([P, ELEM], MD, tag="fb")
                    nc.scalar.mul(fb[:, 0:D], xb[:], ncol[:])
                    nc.vector.memset(fb[:, D : D + 1], 1.0)
                    nc.vector.memset(fb[:, D + 1 : ELEM], 0.0)
                    nc.sync.dma_start(
                        out=feat_s[P * b : P * b + nb, :], in_=fb[:nb, :]
                    )

                # AllGather piece p covers shard rows [PSTART[p], PSTART[p]+PSZ[p]);
                # issued as soon as its last 128-row block is written, overlapping
                # the collective with the rest of phase 1.
                piece_end_blk = {
                    (PSTART[p] + PSZ[p] + P - 1) // P - 1: p for p in range(NSEG)
                }

                def emit_allgather(p):
                    with tc.tile_critical():
                        nc.gpsimd.collective_compute(
                            "AllGather",
                            mybir.AluOpType.bypass,
                            replica_groups=[list(range(N_CORES))],
                            ins=[feat_s[PSTART[p] : PSTART[p] + PSZ[p], :]],
                            outs=[feat_fp[p][:]],
                        ).then_inc(ccsem, 1)

                def maybe_allgather(b):
                    p = piece_end_blk.get(b)
                    if p is not None:
                        emit_allgather(p)

                for t0, cw, wins in meta["p1_chunks"]:
                    wt = p_win.tile([P, CHUNK1], MD, tag="wt")
                    nc.sync.dma_start(out=wt[:, :cw], in_=p1win_d[:, t0 : t0 + cw])
                    oh = p_oh.tile([P, W1, CHUNK1], MD, tag="oh")
                    nc.vector.tensor_tensor(
                        out=oh[:, :, :cw],
                        in0=wt[:, None, :cw].to_broadcast([P, W1, cw]),
                        in1=iota1_sb[:, :, None].to_broadcast([P, W1, cw]),
                        op=mybir.AluOpType.is_equal,
                    )
                    for w, a, bnd in wins:
                        S = p_s.tile([P, W1, 1], MD, tag="S")
                        with nc.allow_low_precision(
                            reason="one-hot counts <=64 are exact in bf16"
                        ):
                            nc.vector.tensor_reduce(
                                out=S[:],
                                in_=oh[:, :, a:bnd],
                                axis=mybir.AxisListType.X,
                                op=mybir.AluOpType.add,
                            )
                        b, j = w // 4, w % 4
                        if b not in ps_blk:
                            ps_blk[b] = p_ps.tile([1, P], F32, name="psblk", tag="psblk")
                        nc.tensor.matmul(
                            out=ps_blk[b][:, W1 * j : W1 * (j + 1)],
                            lhsT=onesm_sb[:],
                            rhs=S[:, :, 0],
                            start=True,
                            stop=True,
                        )
                        if w == meta["p1_last_win_of_blk"].get(b, -1):
                            p1_block_epilogue(b, ps_blk.pop(b))
                            maybe_allgather(b)

            # -------- phase 2: batched gather + scatter matmul + W ----------
            with (
                tc.tile_pool(name="p2i", bufs=2) as p_idx,
                tc.tile_pool(name="p2w", bufs=2) as p_win2,
                tc.tile_pool(name="p2g", bufs=3) as p_g,
                tc.tile_pool(name="p2oh", bufs=3) as p_oh2,
                tc.tile_pool(name="p2ps", bufs=3, space="PSUM") as p_ps2,
                tc.tile_pool(name="p2tr", bufs=2, space="PSUM") as p_tr2,
                tc.tile_pool(name="p2ops", bufs=2, space="PSUM") as p_ops,
                tc.tile_pool(name="p2mrg", bufs=3) as p_mrg,
                tc.tile_pool(name="p2out", bufs=2) as p_out,
                tc.tile_pool(name="p2misc", bufs=4) as p_misc2,
            ):
                nreg = {}
                for sc in sc_list:
                    for q, lt0, n in sc["calls"]:
                        if n * P not in nreg:
                            nreg[n * P] = nc.gpsimd.to_reg(n * P)
                gcall_i = 0
                seg_waited = set()
                for sc in sc_list:
                    nt = sc["nt"]
                    if nt == 0:
                        continue
                    t0 = sc["t0"]
                    ix = p_idx.tile([P, nt_max * 8], I16, tag="ix")
                    nc.sync.dma_start(
                        out=ix[:, : nt * 8], in_=p2idx_d[:, t0 * 8 : (t0 + nt) * 8]
                    )
                    wt = p_win2.tile([P, nt_max], MD, tag="wt2")
                    nc.sync.dma_start(out=wt[:, :nt], in_=p2win_d[:, t0 : t0 + nt])
                    gd = p_g.tile([P, nt_max, ELEM], MD, tag="gd")
                    oh = p_oh2.tile([P, nt_max, W2], MD, tag="oh2")
                    for q, lt0, n in sc["calls"]:
                        if q not in seg_waited:
                            with tc.tile_critical():
                                nc.gpsimd.wait_ge(ccsem, q + 1)
                            seg_waited.add(q)
                        nc.gpsimd.dma_gather(
                            gd[:, lt0 : lt0 + n, :],
                            feat_fp[q][:],
                            ix[:, lt0 * 8 : (lt0 + n) * 8],
                            n * P,
                            nreg[n * P],
                            ELEM,
                            queue_num=gcall_i % NSWQ,
                        )
                        gcall_i += 1
                        nc.vector.tensor_tensor(
                            out=oh[:, lt0 : lt0 + n, :],
                            in0=wt[:, lt0 : lt0 + n, None].to_broadcast([P, n, W2]),
                            in1=iota2_sb[:, None, :].to_broadcast([P, n, W2]),
                            op=mybir.AluOpType.is_equal,
                        )
                    for w in sc["ws"]:
                        lts = sc["wtiles"][w]
                        if not lts:
                            continue
                        ps = p_ps2.tile([D + 1, W2], F32, tag="ps2")
                        for i, lt in enumerate(lts):
                            nc.tensor.matmul(
                                out=ps[:],
                                lhsT=gd[:, lt, 0 : D + 1],
                                rhs=oh[:, lt, :],
                                start=(i == 0),
                                stop=(i == len(lts) - 1),
                            )
                        # norm_dst from the exact deg row, via transpose
                        rowc = p_misc2.tile([1, P], F32, tag="d_row")
                        nc.vector.tensor_copy(rowc[:], ps[D : D + 1, :])
                        tp2 = p_tr2.tile([P, 1], F32)
                        nc.tensor.matmul(
                            out=tp2[:],
                            lhsT=rowc[:],
                            rhs=ones_sb[0:1, 0:1],
                            start=True,
                            stop=True,
                        )
                        dcl = p_misc2.tile([P, 1], F32, tag="d_dcl")
                        nc.vector.tensor_scalar_max(dcl[:], tp2[:], 1.0)
                        dsq = p_misc2.tile([P, 1], F32, tag="d_dsq")
                        nc.scalar.sqrt(dsq[:], dcl[:])
                        drr = p_misc2.tile([P, 1], F32, tag="d_drr")
                        nc.vector.reciprocal(drr[:], dsq[:])
                        ag = p_mrg.tile([D, P], MD, tag="agf")
                        nc.scalar.copy(ag[:], ps[0:D, :])
                        op = p_ops.tile([P, D], F32)
                        nc.tensor.matmul(
                            out=op[:],
                            lhsT=ag[:],
                            rhs=w64_sb[:],
                            start=True,
                            stop=True,
                        )
                        ob = p_out.tile([P, D], F32, tag="ob")
                        nc.scalar.mul(ob[:], op[:], drr[:])
                        nb = min(P, SHARD - W2 * w)
                        nc.sync.dma_start(
                            out=out_d[W2 * w : W2 * w + nb, :], in_=ob[:nb, :]
                        )
                # windows with no edges anywhere: write zeros
                for w in range(nwin2):
                    if w not in win_first:
                        zb = p_out.tile([P, D], F32, tag="ob")
                        nc.vector.memset(zb[:], 0.0)
                        nb = min(P, SHARD - W2 * w)
                        nc.sync.dma_start(
                            out=out_d[W2 * w : W2 * w + nb, :], in_=zb[:nb, :]
                        )

    if do_split_waits:
        split_waits(nc)
    hoist_library_reload(nc)
    mybir.codegen_inst_isa_subclasses(nc)
    return nc


def kernel(x, W, src, dst):
    from concourse.bass_utils import run_bass_kernel_spmd

    ins_maps, meta = _prep(x, W, src, dst)
    meta = _tile_maps(meta)
    nc = _build_nc(meta)
    res = run_bass_kernel_spmd(nc, ins_maps, list(range(N_CORES)))
    out = np.concatenate([res.results[k]["out"] for k in range(N_CORES)], axis=0)
    return out.astype(np.float32)


# revision 51
# speedup vs baseline: 1.2276x; 1.0147x over previous
"""GCNConv (N=100000 nodes, d=64, E=1.6M edges) on 8 Trainium2 NeuronCores.

Formula (DGL GraphConv, in==out feats):
    out_deg = bincount(src); in_deg = bincount(dst)
    norm_src = clip(out_deg,1)^-0.5 ; norm_dst = clip(in_deg,1)^-0.5
    feat = x * norm_src[:,None]
    agg[d] = sum_{e: dst[e]=d} feat[src[e]]
    out = (agg * norm_dst[:,None]) @ W

Distribution: nodes sharded 8 ways (12500/core).
  Phase 1 (core k, edges with src in shard k): out-degree histogram over
    32-node windows (DVE one-hot + free-axis reduce + tiny count matmul into
    a PSUM degree row per 128-node block); per block: PE-transpose the raw
    degree row to a column, clip/sqrt(ACT)/recip on [128,1], ACT row-scale
    the x block, write feat shard [12500, 128] bf16 (col 64 = 1.0 ->
    in-degree for free in phase 2; cols 65..127 zero pad to a 256B row for
    the SWDGE gather).
  AllGather in 4 pieces (one per block-aligned shard slice; piece p doubles
    as int16 gather segment p of <=25600 rows). Each piece is issued from a
    tile_critical as soon as its blocks are written, so collectives overlap
    the remainder of phase 1; phase-2 gathers gate on ccsem >= p+1.
  Phase 2 (core k, edges with dst in shard k): edges bucketed by
    (128-node dst window, segment); superchunks of GW=4 windows. Tiles of
    128 edges are gathered in batches of up to 8 tiles (1024 rows) with ONE
    gpsimd.dma_gather per batch, round-robined over 4 SWDGE queues (the
    994ns fixed SWDGE cost is amortized 8x and the 1024-descriptor ring
    drains overlap across queues; a single queue serializes). Per tile: a
    one-hot scatter matmul accumulates into a per-window single-bank PSUM
    tile [65, 128] (row 64 = in_deg). Windows accumulate strictly
    window-major: interleaving accumulation regions within a PSUM bank
    corrupts results (start appears to arm per bank, not per region).
    Per window: norm_dst via deg-row transpose -> [128,1] clip/sqrt/recip,
    agg copied to SBUF bf16 on ACT, out_blk = agg^T @ W, ACT row-scale,
    store.

Host side only shards/buckets edges and builds index/window inputs; all
arithmetic of the formula (degrees, norms, scaling, aggregation, matmul)
runs on device.

Perf journey (HW exec): 3084us baseline (per-tile indirect DMA, gpsimd
desc-gen bound) -> 1284us (batched dma_gather, 4 queues) -> 1234us
(pieced AllGather overlap) -> 1146us (ACT offload of row-scales/copies).
A 2-pass segment-split (start gathers before phase 1 ends) was tried and
reverted: halved per-superchunk buffers shallow the gather pipeline and
cost more than the earlier start gained.
"""

import sys

if "/opt/trn_rl_repo" not in sys.path:
    sys.path.insert(0, "/opt/trn_rl_repo")

import numpy as np

import concourse.bass as bass
import concourse.mybir as mybir
import concourse.tile as tile
from concourse.library_config import mlp as _mlp_lib

N_NODES = 100000
D = 64
N_CORES = 8
SHARD = N_NODES // N_CORES  # 12500
W1 = 32  # phase-1 (degree-count) window width
W2 = 128  # phase-2 dst window == node block
P = 128  # edges per tile (matmul contraction dim)
CHUNK1 = 64  # phase-1 max tiles per chunk (window-aligned packing)
ELEM = 128  # gather row width in bf16 (256 B)
NSEG = 4  # int16 gather table segments == AllGather pieces
PSTART = [0, 3200, 6400, 9600]  # piece starts within a shard (block-aligned)
PSZ = [3200, 3200, 3200, 2900]  # piece sizes; table_p = 8*PSZ[p] rows < 2**15
GW = 4  # dst windows per superchunk (PSUM block [65, GW*128] = 1 bank)
MAXG = 8  # tiles per dma_gather (8*128 = 1024 rows; HW ring caps ~1024)
NSWQ = 4  # SWDGE queues; gather calls round-robin across them

F32 = mybir.dt.float32
BF16 = mybir.dt.bfloat16
I16 = mybir.dt.int16

MD = BF16


def split_waits(nc, maxw=1):
    """This walrus build allows at most `maxw` sem-waits per instruction;
    move extras onto preceding InstEventSemaphore carriers (same engine)."""
    for f in nc.m.functions:
        for blk in f.blocks:
            newl = []
            for ins in blk.instructions:
                si = ins.sync_info
                if si is not None and si.on_wait and len(si.on_wait) > maxw:
                    waits = list(si.on_wait)
                    carry, keep = waits[:-maxw], waits[-maxw:]
                    for i in range(0, len(carry), maxw):
                        w = mybir.InstEventSemaphore(
                            name=nc.get_next_instruction_name(), ins=[], outs=[]
                        )
                        w.engine = ins.engine
                        w.sync_info = mybir.SyncInfo(
                            on_wait=carry[i : i + maxw], on_update=[]
                        )
                        newl.append(w)
                    ins.sync_info = mybir.SyncInfo(
                        on_wait=keep, on_update=list(si.on_update)
                    )
                newl.append(ins)
            blk.instructions[:] = newl


def hoist_library_reload(nc):
    """Move the gpsimd library-reload pseudo inst ahead of the first Pool
    instruction so the mlp ucode (dma_gather) is resident before use."""
    import concourse.bass_isa as bass_isa

    for f in nc.m.functions:
        for blk in f.blocks:
            insts = blk.instructions
            ri = next(
                (
                    i
                    for i, ins in enumerate(insts)
                    if isinstance(ins, bass_isa.InstPseudoReloadLibraryIndex)
                ),
                None,
            )
            if ri is None:
                continue
            pi = next(
                (
                    i
                    for i, ins in enumerate(insts)
                    if ins.engine == mybir.EngineType.Pool
                    and not isinstance(ins, bass_isa.InstPseudoReloadLibraryIndex)
                ),
                None,
            )
            if pi is not None and pi < ri:
                reload = insts.pop(ri)
                insts.insert(pi, reload)


def _layout(cnts_per_core):
    """Uniform (max-over-cores) tiles per window."""
    tiles_w = (cnts_per_core.max(axis=0) + P - 1) // P
    tbase = np.concatenate([[0], np.cumsum(tiles_w)[:-1]])
    return tiles_w.astype(np.int64), tbase.astype(np.int64), int(tiles_w.sum())


def _prep(x, W, src, dst):
    """Host-side sharding: bucket edges by shard/window/segment, build
    per-core device inputs and the shared (uniform) tile metadata."""
    import ml_dtypes

    src = np.asarray(src)
    dst = np.asarray(dst)
    x = np.asarray(x, dtype=np.float32)
    W = np.asarray(W, dtype=np.float32)

    nwin1 = (SHARD + W1 - 1) // W1
    nwin2 = (SHARD + W2 - 1) // W2

    per_core = []
    c1 = np.zeros((N_CORES, nwin1), dtype=np.int64)
    c2 = np.zeros((N_CORES, nwin2 * NSEG), dtype=np.int64)
    for k in range(N_CORES):
        sel1 = (src // SHARD) == k
        loc1 = src[sel1] - SHARD * k
        w1v = loc1 // W1
        c1[k] = np.bincount(w1v, minlength=nwin1)

        sel2 = (dst // SHARD) == k
        loc2 = dst[sel2] - SHARD * k
        gidx = src[sel2].astype(np.int64)
        wv = loc2 // W2
        slot = (loc2 % W2).astype(np.float32)
        gs = gidx // SHARD  # owning shard of the src node
        off = gidx - gs * SHARD
        qv = np.minimum(off // 3200, NSEG - 1)  # AllGather piece == segment
        lidx = gs * np.asarray(PSZ)[qv] + (off - np.asarray(PSTART)[qv])
        key = wv * NSEG + qv
        c2[k] = np.bincount(key, minlength=nwin2 * NSEG)
        per_core.append((loc1, w1v, key, slot, lidx))

    t1_w, t1_base, T1 = _layout(c1)

    # ---- phase-2 layout: superchunks of GW windows, quarter-major inside ----
    t2_wq = ((c2.max(axis=0) + P - 1) // P).astype(np.int64)  # [nwin2*NSEG]
    tile_base = np.zeros(nwin2 * NSEG, dtype=np.int64)
    sc_list = []
    win_first = {}
    win_last = {}
    t = 0
    for w0 in range(0, nwin2, GW):
        ws = list(range(w0, min(w0 + GW, nwin2)))
        sc = {"w0": w0, "ws": ws, "t0": t, "wtiles": {w: [] for w in ws}, "calls": []}
        for q in range(NSEG):
            run_t0 = t
            for w in ws:
                keyi = w * NSEG + q
                n = int(t2_wq[keyi])
                if n == 0:
                    continue
                tile_base[keyi] = t
                for _ in range(n):
                    if w not in win_first:
                        win_first[w] = t
                    win_last[w] = t
                    sc["wtiles"][w].append(t - sc["t0"])
                    t += 1
            nrun = t - run_t0
            o = 0
            while o < nrun:
                n = min(MAXG, nrun - o)
                sc["calls"].append((q, run_t0 - sc["t0"] + o, n))
                o += n
        sc["nt"] = t - sc["t0"]
        # pass A = segments 0-1 (a prefix of the sc's tiles), pass B = 2-3
        sc["nA"] = sum(
            int(t2_wq[w * NSEG + q]) for q in range(NSEG // 2) for w in ws
        )
        sc["wtiles_A"] = {w: [lt for lt in sc["wtiles"][w] if lt < sc["nA"]] for w in ws}
        sc["wtiles_B"] = {w: [lt for lt in sc["wtiles"][w] if lt >= sc["nA"]] for w in ws}
        sc["calls_A"] = [c for c in sc["calls"] if c[0] < NSEG // 2]
        sc["calls_B"] = [c for c in sc["calls"] if c[0] >= NSEG // 2]
        sc_list.append(sc)
    T2 = t

    bf16 = ml_dtypes.bfloat16
    iota1 = np.broadcast_to(np.arange(W1, dtype=np.float32), (P, W1)).astype(bf16)
    iota2 = np.broadcast_to(np.arange(W2, dtype=np.float32), (P, W2)).astype(bf16)
    ones = np.ones((P, 1), dtype=np.float32)
    ones_m = np.ones((P, 1), dtype=bf16)
    ident = np.eye(D + 1, dtype=np.float32).astype(bf16)
    w64 = W.astype(bf16)

    ins_maps = []
    for k in range(N_CORES):
        loc1, w1v, key, slot, lidx = per_core[k]

        # phase-1 window map (as before)
        order1 = np.argsort(w1v, kind="stable")
        ws1 = w1v[order1]
        cnt1 = np.bincount(w1v, minlength=nwin1)
        starts1 = np.concatenate([[0], np.cumsum(cnt1)[:-1]])
        rank1 = np.arange(len(order1)) - starts1[ws1]
        col1 = t1_base[ws1] + rank1 // P
        lane1 = rank1 % P
        p1win = np.full((P, T1), float(W1), dtype=np.float32)
        p1win[lane1, col1] = (loc1[order1] - W1 * ws1).astype(np.float32)
        p1win = p1win.astype(bf16)

        # phase-2: slot codes + wrapped int16 gather indices
        order = np.argsort(key, kind="stable")
        ks = key[order]
        cnt = np.bincount(key, minlength=nwin2 * NSEG)
        starts = np.concatenate([[0], np.cumsum(cnt)[:-1]])
        rank = np.arange(len(order)) - starts[ks]
        tau = tile_base[ks] + rank // P
        lane = rank % P
        p2win = np.full((P, T2), float(W2), dtype=np.float32)
        p2win[lane, tau] = slot[order]
        p2win = p2win.astype(bf16)
        idx16 = np.zeros((16, T2 * 8), dtype=np.int16)
        idx16[lane % 16, tau * 8 + lane // 16] = lidx[order].astype(np.int16)
        p2idx = np.tile(idx16, (8, 1))

        ins_maps.append(
            {
                "xs": np.ascontiguousarray(x[SHARD * k : SHARD * (k + 1)]),
                "p1win": p1win,
                "p2win": p2win,
                "p2idx": p2idx,
                "w64": w64,
                "iota1": iota1,
                "iota2": iota2,
                "ones": ones,
                "ones_m": ones_m,
                "ident": ident,
            }
        )

    meta = {
        "T1": T1,
        "T2": T2,
        "t1_w": t1_w,
        "nwin1": nwin1,
        "nwin2": nwin2,
        "sc_list": sc_list,
        "win_first": win_first,
        "win_last": win_last,
    }
    return ins_maps, meta


def _tile_maps(meta):
    # phase-1: pack whole windows into chunks of <= CHUNK1 tiles.
    chunks1 = []
    cur = []
    t0 = 0
    pos = 0
    for w, n in enumerate(meta["t1_w"]):
        n = int(n)
        if n == 0:
            continue
        if pos + n > CHUNK1 and cur:
            chunks1.append((t0, pos, cur))
            t0 += pos
            pos = 0
            cur = []
        cur.append((w, pos, pos + n))
        pos += n
    if cur:
        chunks1.append((t0, pos, cur))
    meta["p1_chunks"] = chunks1
    last_win_of_blk = {}
    for w, n in enumerate(meta["t1_w"]):
        if int(n) > 0:
            last_win_of_blk[w // 4] = w
    meta["p1_last_win_of_blk"] = last_win_of_blk
    return meta


def _build_nc(meta, do_split_waits=True, dbg=False):
    T1, T2 = meta["T1"], meta["T2"]
    t1_w = meta["t1_w"]
    nwin2 = meta["nwin2"]
    sc_list = meta["sc_list"]
    win_first = meta["win_first"]
    win_last = meta["win_last"]
    nt_max = max(sc["nt"] for sc in sc_list)

    nc = bass.Bass(num_swdge_queues=NSWQ)
    xs = nc.declare_dram_parameter("xs", [SHARD, D], F32, isOutput=False)
    p1win_d = nc.declare_dram_parameter("p1win", [P, T1], MD, isOutput=False)
    p2win_d = nc.declare_dram_parameter("p2win", [P, T2], MD, isOutput=False)
    p2idx_d = nc.declare_dram_parameter("p2idx", [P, T2 * 8], I16, isOutput=False)
    w64_d = nc.declare_dram_parameter("w64", [D, D], MD, isOutput=False)
    iota1_d = nc.declare_dram_parameter("iota1", [P, W1], MD, isOutput=False)
    iota2_d = nc.declare_dram_parameter("iota2", [P, W2], MD, isOutput=False)
    ones_d = nc.declare_dram_parameter("ones", [P, 1], F32, isOutput=False)
    onesm_d = nc.declare_dram_parameter("ones_m", [P, 1], MD, isOutput=False)
    ident_d = nc.declare_dram_parameter("ident", [D + 1, D + 1], MD, isOutput=False)
    out_d = nc.declare_dram_parameter("out", [SHARD, D], F32, isOutput=True)

    feat_s = nc.dram_tensor("feat_s", [SHARD, ELEM], MD)
    feat_fp = [
        nc.dram_tensor(f"feat_f{p}", [N_CORES * PSZ[p], ELEM], MD)
        for p in range(NSEG)
    ]
    if dbg:
        nt0 = sc_list[0]["nt"]
        dbg_gd_d = nc.declare_dram_parameter("dbg_gd", [P, nt0, ELEM], MD, isOutput=True)
        dbg_oh_d = nc.declare_dram_parameter("dbg_oh", [P, nt0, W2], MD, isOutput=True)

    with tile.TileContext(nc) as tc:
        with tc.tile_pool(name="consts", bufs=1) as consts:
            nc.gpsimd.load_library(_mlp_lib)
            w64_sb = consts.tile([D, D], MD, tag="w64")
            iota1_sb = consts.tile([P, W1], MD, tag="iota1")
            iota2_sb = consts.tile([P, W2], MD, tag="iota2")
            ones_sb = consts.tile([P, 1], F32, tag="ones")
            onesm_sb = consts.tile([P, 1], MD, tag="onesm")
            ident_sb = consts.tile([D + 1, D + 1], MD, tag="ident")
            nc.sync.dma_start(out=w64_sb[:], in_=w64_d[:])
            nc.sync.dma_start(out=iota1_sb[:], in_=iota1_d[:])
            nc.sync.dma_start(out=iota2_sb[:], in_=iota2_d[:])
            nc.sync.dma_start(out=ones_sb[:], in_=ones_d[:])
            nc.sync.dma_start(out=onesm_sb[:], in_=onesm_d[:])
            nc.sync.dma_start(out=ident_sb[:], in_=ident_d[:])
            ccsem = nc.alloc_semaphore("ccsem")

            # ---------------- phase 1: out-degree -> feat shard -------------
            with (
                tc.tile_pool(name="p1win", bufs=2) as p_win,
                tc.tile_pool(name="p1oh", bufs=2) as p_oh,
                tc.tile_pool(name="p1s", bufs=4) as p_s,
                tc.tile_pool(name="p1ps", bufs=2, space="PSUM") as p_ps,
                tc.tile_pool(name="p1trps", bufs=2, space="PSUM") as p_trps,
                tc.tile_pool(name="p1x", bufs=2) as p_x,
                tc.tile_pool(name="p1feat", bufs=2) as p_feat,
                tc.tile_pool(name="p1misc", bufs=4) as p_misc,
            ):
                ps_blk = {}

                def p1_block_epilogue(b, ps):
                    for j2 in range(4):
                        w2 = 4 * b + j2
                        if w2 >= meta["nwin1"] or t1_w[w2] == 0:
                            nc.vector.memset(ps[:, W1 * j2 : W1 * (j2 + 1)], 0.0)
                    # raw degree row [1,128] -> SBUF -> PE transpose -> [128,1]
                    rowc = p_misc.tile([1, P], F32, tag="m_row")
                    nc.vector.tensor_copy(rowc[:], ps[:])
                    tp = p_trps.tile([P, 1], F32)
                    nc.tensor.matmul(
                        out=tp[:],
                        lhsT=rowc[:],
                        rhs=ones_sb[0:1, 0:1],
                        start=True,
                        stop=True,
                    )
                    dcl = p_misc.tile([P, 1], F32, tag="m_dcl")
                    nc.vector.tensor_scalar_max(dcl[:], tp[:], 1.0)
                    dsq = p_misc.tile([P, 1], F32, tag="m_dsq")
                    nc.scalar.sqrt(dsq[:], dcl[:])
                    ncol = p_misc.tile([P, 1], F32, tag="m_ncol")
                    nc.vector.reciprocal(ncol[:], dsq[:])
                    nb = min(P, SHARD - P * b)
                    xb = p_x.tile([P, D], F32, tag="xb")
                    nc.sync.dma_start(out=xb[:nb], in_=xs[P * b : P * b + nb, :])
                    fb = p_feat.tile([P, ELEM], MD, tag="fb")
                    nc.scalar.mul(fb[:, 0:D], xb[:], ncol[:])
                    nc.vector.memset(fb[:, D : D + 1], 1.0)
                    nc.vector.memset(fb[:, D + 1 : ELEM], 0.0)
                    nc.sync.dma_start(
                        out=feat_s[P * b : P * b + nb, :], in_=fb[:nb, :]
                    )

                # AllGather piece p covers shard rows [PSTART[p], PSTART[p]+PSZ[p]);
                # issued as soon as its last 128-row block is written, overlapping
                # the collective with the rest of phase 1.
                piece_end_blk = {
                    (PSTART[p] + PSZ[p] + P - 1) // P - 1: p for p in range(NSEG)
                }

                def emit_allgather(p):
                    with tc.tile_critical():
                        nc.gpsimd.collective_compute(
                            "AllGather",
                            mybir.AluOpType.bypass,
                            replica_groups=[list(range(N_CORES))],
                            ins=[feat_s[PSTART[p] : PSTART[p] + PSZ[p], :]],
                            outs=[feat_fp[p][:]],
                        ).then_inc(ccsem, 1)

                def maybe_allgather(b):
                    p = piece_end_blk.get(b)
                    if p is not None:
                        emit_allgather(p)

                for t0, cw, wins in meta["p1_chunks"]:
                    wt = p_win.tile([P, CHUNK1], MD, tag="wt")
                    nc.sync.dma_start(out=wt[:, :cw], in_=p1win_d[:, t0 : t0 + cw])
                    oh = p_oh.tile([P, W1, CHUNK1], MD, tag="oh")
                    nc.vector.tensor_tensor(
                        out=oh[:, :, :cw],
                        in0=wt[:, None, :cw].to_broadcast([P, W1, cw]),
                        in1=iota1_sb[:, :, None].to_broadcast([P, W1, cw]),
                        op=mybir.AluOpType.is_equal,
                    )
                    for w, a, bnd in wins:
                        S = p_s.tile([P, W1, 1], MD, tag="S")
                        with nc.allow_low_precision(
                            reason="one-hot counts <=64 are exact in bf16"
                        ):
                            nc.vector.tensor_reduce(
                                out=S[:],
                                in_=oh[:, :, a:bnd],
                                axis=mybir.AxisListType.X,
                                op=mybir.AluOpType.add,
                            )
                        b, j = w // 4, w % 4
                        if b not in ps_blk:
                            ps_blk[b] = p_ps.tile([1, P], F32, name="psblk", tag="psblk")
                        nc.tensor.matmul(
                            out=ps_blk[b][:, W1 * j : W1 * (j + 1)],
                            lhsT=onesm_sb[:],
                            rhs=S[:, :, 0],
                            start=True,
                            stop=True,
                        )
                        if w == meta["p1_last_win_of_blk"].get(b, -1):
                            p1_block_epilogue(b, ps_blk.pop(b))
                            maybe_allgather(b)

            # -------- phase 2: batched gather + scatter matmul + W ----------
            with (
                tc.tile_pool(name="p2i", bufs=2) as p_idx,
                tc.tile_pool(name="p2w", bufs=2) as p_win2,
                tc.tile_pool(name="p2g", bufs=4) as p_g,
                tc.tile_pool(name="p2oh", bufs=3) as p_oh2,
                tc.tile_pool(name="p2ps", bufs=3, space="PSUM") as p_ps2,
                tc.tile_pool(name="p2tr", bufs=2, space="PSUM") as p_tr2,
                tc.tile_pool(name="p2ops", bufs=2, space="PSUM") as p_ops,
                tc.tile_pool(name="p2mrg", bufs=3) as p_mrg,
                tc.tile_pool(name="p2out", bufs=2) as p_out,
                tc.tile_pool(name="p2misc", bufs=4) as p_misc2,
            ):
                nreg = {}
                for sc in sc_list:
                    for q, lt0, n in sc["calls"]:
                        if n * P not in nreg:
                            nreg[n * P] = nc.gpsimd.to_reg(n * P)
                gcall_i = 0
                seg_waited = set()
                for sc in sc_list:
                    nt = sc["nt"]
                    if nt == 0:
                        continue
                    t0 = sc["t0"]
                    ix = p_idx.tile([P, nt_max * 8], I16, tag="ix")
                    nc.sync.dma_start(
                        out=ix[:, : nt * 8], in_=p2idx_d[:, t0 * 8 : (t0 + nt) * 8]
                    )
                    wt = p_win2.tile([P, nt_max], MD, tag="wt2")
                    nc.sync.dma_start(out=wt[:, :nt], in_=p2win_d[:, t0 : t0 + nt])
                    gd = p_g.tile([P, nt_max, ELEM], MD, tag="gd")
                    oh = p_oh2.tile([P, nt_max, W2], MD, tag="oh2")
                    for q, lt0, n in sc["calls"]:
                        if q not in seg_waited:
                            with tc.tile_critical():
                                nc.gpsimd.wait_ge(ccsem, q + 1)
                            seg_waited.add(q)
                        nc.gpsimd.dma_gather(
                            gd[:, lt0 : lt0 + n, :],
                            feat_fp[q][:],
                            ix[:, lt0 * 8 : (lt0 + n) * 8],
                            n * P,
                            nreg[n * P],
                            ELEM,
                            queue_num=gcall_i % NSWQ,
                        )
                        gcall_i += 1
                        nc.vector.tensor_tensor(
                            out=oh[:, lt0 : lt0 + n, :],
                            in0=wt[:, lt0 : lt0 + n, None].to_broadcast([P, n, W2]),
                            in1=iota2_sb[:, None, :].to_broadcast([P, n, W2]),
                            op=mybir.AluOpType.is_equal,
                        )
                    for w in sc["ws"]:
                        lts = sc["wtiles"][w]
                        if not lts:
                            continue
                        ps = p_ps2.tile([D + 1, W2], F32, tag="ps2")
                        for i, lt in enumerate(lts):
                            nc.tensor.matmul(
                                out=ps[:],
                                lhsT=gd[:, lt, 0 : D + 1],
                                rhs=oh[:, lt, :],
                                start=(i == 0),
                                stop=(i == len(lts) - 1),
                            )
                        # norm_dst from the exact deg row, via transpose
                        rowc = p_misc2.tile([1, P], F32, tag="d_row")
                        nc.vector.tensor_copy(rowc[:], ps[D : D + 1, :])
                        tp2 = p_tr2.tile([P, 1], F32)
                        nc.tensor.matmul(
                            out=tp2[:],
                            lhsT=rowc[:],
                            rhs=ones_sb[0:1, 0:1],
                            start=True,
                            stop=True,
                        )
                        dcl = p_misc2.tile([P, 1], F32, tag="d_dcl")
                        nc.vector.tensor_scalar_max(dcl[:], tp2[:], 1.0)
                        dsq = p_misc2.tile([P, 1], F32, tag="d_dsq")
                        nc.scalar.sqrt(dsq[:], dcl[:])
                        drr = p_misc2.tile([P, 1], F32, tag="d_drr")
                        nc.vector.reciprocal(drr[:], dsq[:])
                        ag = p_mrg.tile([D, P], MD, tag="agf")
                        nc.scalar.copy(ag[:], ps[0:D, :])
                        op = p_ops.tile([P, D], F32)
                        nc.tensor.matmul(
                            out=op[:],
                            lhsT=ag[:],
                            rhs=w64_sb[:],
                            start=True,
                            stop=True,
                        )
                        ob = p_out.tile([P, D], F32, tag="ob")
                        nc.scalar.mul(ob[:], op[:], drr[:])
                        nb = min(P, SHARD - W2 * w)
                        nc.sync.dma_start(
                            out=out_d[W2 * w : W2 * w + nb, :], in_=ob[:nb, :]
                        )
                # windows with no edges anywhere: write zeros
                for w in range(nwin2):
                    if w not in win_first:
                        zb = p_out.tile([P, D], F32, tag="ob")
                        nc.vector.memset(zb[:], 0.0)
                        nb = min(P, SHARD - W2 * w)
                        nc.sync.dma_start(
                            out=out_d[W2 * w : W2 * w + nb, :], in_=zb[:nb, :]
                        )

    if do_split_waits:
        split_waits(nc)
    hoist_library_reload(nc)
    mybir.codegen_inst_isa_subclasses(nc)
    return nc


def kernel(x, W, src, dst):
    from concourse.bass_utils import run_bass_kernel_spmd

    ins_maps, meta = _prep(x, W, src, dst)
    meta = _tile_maps(meta)
    nc = _build_nc(meta)
    res = run_bass_kernel_spmd(nc, ins_maps, list(range(N_CORES)))
    out = np.concatenate([res.results[k]["out"] for k in range(N_CORES)], axis=0)
    return out.astype(np.float32)


# revision 53
# speedup vs baseline: 1.2316x; 1.0032x over previous
"""GCNConv (N=100000 nodes, d=64, E=1.6M edges) on 8 Trainium2 NeuronCores.

Formula (DGL GraphConv, in==out feats):
    out_deg = bincount(src); in_deg = bincount(dst)
    norm_src = clip(out_deg,1)^-0.5 ; norm_dst = clip(in_deg,1)^-0.5
    feat = x * norm_src[:,None]
    agg[d] = sum_{e: dst[e]=d} feat[src[e]]
    out = (agg * norm_dst[:,None]) @ W

Distribution: nodes sharded 8 ways (12500/core).
  Phase 1 (core k, edges with src in shard k): out-degree histogram over
    32-node windows (DVE one-hot + free-axis reduce + tiny count matmul into
    a PSUM degree row per 128-node block); per block: PE-transpose the raw
    degree row to a column, clip/sqrt(ACT)/recip on [128,1], ACT row-scale
    the x block, write feat shard [12500, 128] bf16 (col 64 = 1.0 ->
    in-degree for free in phase 2; cols 65..127 zero pad to a 256B row for
    the SWDGE gather).
  AllGather in 4 pieces (one per block-aligned shard slice; piece p doubles
    as int16 gather segment p of <=25600 rows). Each piece is issued from a
    tile_critical as soon as its blocks are written, so collectives overlap
    the remainder of phase 1; phase-2 gathers gate on ccsem >= p+1.
  Phase 2 (core k, edges with dst in shard k): edges bucketed by
    (128-node dst window, segment); superchunks of GW=4 windows. Tiles of
    128 edges are gathered in batches of up to 8 tiles (1024 rows) with ONE
    gpsimd.dma_gather per batch, round-robined over 4 SWDGE queues (the
    994ns fixed SWDGE cost is amortized 8x and the 1024-descriptor ring
    drains overlap across queues; a single queue serializes). Per tile: a
    one-hot scatter matmul accumulates into a per-window single-bank PSUM
    tile [65, 128] (row 64 = in_deg). Windows accumulate strictly
    window-major: interleaving accumulation regions within a PSUM bank
    corrupts results (start appears to arm per bank, not per region).
    Per window: norm_dst via deg-row transpose -> [128,1] clip/sqrt/recip,
    agg copied to SBUF bf16 on ACT, out_blk = agg^T @ W, ACT row-scale,
    store.

Host side only shards/buckets edges and builds index/window inputs; all
arithmetic of the formula (degrees, norms, scaling, aggregation, matmul)
runs on device.

Perf journey (HW exec): 3084us baseline (per-tile indirect DMA, gpsimd
desc-gen bound) -> 1284us (batched dma_gather, 4 queues) -> 1234us
(pieced AllGather overlap) -> 1146us (ACT offload of row-scales/copies).
A 2-pass segment-split (start gathers before phase 1 ends) was tried and
reverted: halved per-superchunk buffers shallow the gather pipeline and
cost more than the earlier start gained.
"""

import sys

if "/opt/trn_rl_repo" not in sys.path:
    sys.path.insert(0, "/opt/trn_rl_repo")

import numpy as np

import concourse.bass as bass
import concourse.mybir as mybir
import concourse.tile as tile
from concourse.library_config import mlp as _mlp_lib

N_NODES = 100000
D = 64
N_CORES = 8
SHARD = N_NODES // N_CORES  # 12500
W1 = 32  # phase-1 (degree-count) window width
W2 = 128  # phase-2 dst window == node block
P = 128  # edges per tile (matmul contraction dim)
CHUNK1 = 64  # phase-1 max tiles per chunk (window-aligned packing)
ELEM = 128  # gather row width in bf16 (256 B)
NSEG = 4  # int16 gather table segments == AllGather pieces
PSTART = [0, 3200, 6400, 9600]  # piece starts within a shard (block-aligned)
PSZ = [3200, 3200, 3200, 2900]  # piece sizes; table_p = 8*PSZ[p] rows < 2**15
GW = 4  # dst windows per superchunk (PSUM block [65, GW*128] = 1 bank)
MAXG = 8  # tiles per dma_gather (8*128 = 1024 rows; HW ring caps ~1024)
NSWQ = 4  # SWDGE queues; gather calls round-robin across them

F32 = mybir.dt.float32
BF16 = mybir.dt.bfloat16
I16 = mybir.dt.int16

MD = BF16


def split_waits(nc, maxw=1):
    """This walrus build allows at most `maxw` sem-waits per instruction;
    move extras onto preceding InstEventSemaphore carriers (same engine)."""
    for f in nc.m.functions:
        for blk in f.blocks:
            newl = []
            for ins in blk.instructions:
                si = ins.sync_info
                if si is not None and si.on_wait and len(si.on_wait) > maxw:
                    waits = list(si.on_wait)
                    carry, keep = waits[:-maxw], waits[-maxw:]
                    for i in range(0, len(carry), maxw):
                        w = mybir.InstEventSemaphore(
                            name=nc.get_next_instruction_name(), ins=[], outs=[]
                        )
                        w.engine = ins.engine
                        w.sync_info = mybir.SyncInfo(
                            on_wait=carry[i : i + maxw], on_update=[]
                        )
                        newl.append(w)
                    ins.sync_info = mybir.SyncInfo(
                        on_wait=keep, on_update=list(si.on_update)
                    )
                newl.append(ins)
            blk.instructions[:] = newl


def hoist_library_reload(nc):
    """Move the gpsimd library-reload pseudo inst ahead of the first Pool
    instruction so the mlp ucode (dma_gather) is resident before use."""
    import concourse.bass_isa as bass_isa

    for f in nc.m.functions:
        for blk in f.blocks:
            insts = blk.instructions
            ri = next(
                (
                    i
                    for i, ins in enumerate(insts)
                    if isinstance(ins, bass_isa.InstPseudoReloadLibraryIndex)
                ),
                None,
            )
            if ri is None:
                continue
            pi = next(
                (
                    i
                    for i, ins in enumerate(insts)
                    if ins.engine == mybir.EngineType.Pool
                    and not isinstance(ins, bass_isa.InstPseudoReloadLibraryIndex)
                ),
                None,
            )
            if pi is not None and pi < ri:
                reload = insts.pop(ri)
                insts.insert(pi, reload)


def _layout(cnts_per_core):
    """Uniform (max-over-cores) tiles per window."""
    tiles_w = (cnts_per_core.max(axis=0) + P - 1) // P
    tbase = np.concatenate([[0], np.cumsum(tiles_w)[:-1]])
    return tiles_w.astype(np.int64), tbase.astype(np.int64), int(tiles_w.sum())


def _prep(x, W, src, dst):
    """Host-side sharding: bucket edges by shard/window/segment, build
    per-core device inputs and the shared (uniform) tile metadata."""
    import ml_dtypes

    src = np.asarray(src)
    dst = np.asarray(dst)
    x = np.asarray(x, dtype=np.float32)
    W = np.asarray(W, dtype=np.float32)

    nwin1 = (SHARD + W1 - 1) // W1
    nwin2 = (SHARD + W2 - 1) // W2

    per_core = []
    c1 = np.zeros((N_CORES, nwin1), dtype=np.int64)
    c2 = np.zeros((N_CORES, nwin2 * NSEG), dtype=np.int64)
    for k in range(N_CORES):
        sel1 = (src // SHARD) == k
        loc1 = src[sel1] - SHARD * k
        w1v = loc1 // W1
        c1[k] = np.bincount(w1v, minlength=nwin1)

        sel2 = (dst // SHARD) == k
        loc2 = dst[sel2] - SHARD * k
        gidx = src[sel2].astype(np.int64)
        wv = loc2 // W2
        slot = (loc2 % W2).astype(np.float32)
        gs = gidx // SHARD  # owning shard of the src node
        off = gidx - gs * SHARD
        qv = np.minimum(off // 3200, NSEG - 1)  # AllGather piece == segment
        lidx = gs * np.asarray(PSZ)[qv] + (off - np.asarray(PSTART)[qv])
        key = wv * NSEG + qv
        c2[k] = np.bincount(key, minlength=nwin2 * NSEG)
        per_core.append((loc1, w1v, key, slot, lidx))

    t1_w, t1_base, T1 = _layout(c1)

    # ---- phase-2 layout: superchunks of GW windows, quarter-major inside ----
    t2_wq = ((c2.max(axis=0) + P - 1) // P).astype(np.int64)  # [nwin2*NSEG]
    tile_base = np.zeros(nwin2 * NSEG, dtype=np.int64)
    sc_list = []
    win_first = {}
    win_last = {}
    t = 0
    for w0 in range(0, nwin2, GW):
        ws = list(range(w0, min(w0 + GW, nwin2)))
        sc = {"w0": w0, "ws": ws, "t0": t, "wtiles": {w: [] for w in ws}, "calls": []}
        for q in range(NSEG):
            run_t0 = t
            for w in ws:
                keyi = w * NSEG + q
                n = int(t2_wq[keyi])
                if n == 0:
                    continue
                tile_base[keyi] = t
                for _ in range(n):
                    if w not in win_first:
                        win_first[w] = t
                    win_last[w] = t
                    sc["wtiles"][w].append(t - sc["t0"])
                    t += 1
            nrun = t - run_t0
            o = 0
            while o < nrun:
                n = min(MAXG, nrun - o)
                sc["calls"].append((q, run_t0 - sc["t0"] + o, n))
                o += n
        sc["nt"] = t - sc["t0"]
        # pass A = segments 0-1 (a prefix of the sc's tiles), pass B = 2-3
        sc["nA"] = sum(
            int(t2_wq[w * NSEG + q]) for q in range(NSEG // 2) for w in ws
        )
        sc["wtiles_A"] = {w: [lt for lt in sc["wtiles"][w] if lt < sc["nA"]] for w in ws}
        sc["wtiles_B"] = {w: [lt for lt in sc["wtiles"][w] if lt >= sc["nA"]] for w in ws}
        sc["calls_A"] = [c for c in sc["calls"] if c[0] < NSEG // 2]
        sc["calls_B"] = [c for c in sc["calls"] if c[0] >= NSEG // 2]
        sc_list.append(sc)
    T2 = t

    bf16 = ml_dtypes.bfloat16
    iota1 = np.broadcast_to(np.arange(W1, dtype=np.float32), (P, W1)).astype(bf16)
    iota2 = np.broadcast_to(np.arange(W2, dtype=np.float32), (P, W2)).astype(bf16)
    ones = np.ones((P, 1), dtype=np.float32)
    ones_m = np.ones((P, 1), dtype=bf16)
    ident = np.eye(D + 1, dtype=np.float32).astype(bf16)
    w64 = W.astype(bf16)

    ins_maps = []
    for k in range(N_CORES):
        loc1, w1v, key, slot, lidx = per_core[k]

        # phase-1 window map (as before)
        order1 = np.argsort(w1v, kind="stable")
        ws1 = w1v[order1]
        cnt1 = np.bincount(w1v, minlength=nwin1)
        starts1 = np.concatenate([[0], np.cumsum(cnt1)[:-1]])
        rank1 = np.arange(len(order1)) - starts1[ws1]
        col1 = t1_base[ws1] + rank1 // P
        lane1 = rank1 % P
        p1win = np.full((P, T1), float(W1), dtype=np.float32)
        p1win[lane1, col1] = (loc1[order1] - W1 * ws1).astype(np.float32)
        p1win = p1win.astype(bf16)

        # phase-2: slot codes + wrapped int16 gather indices
        order = np.argsort(key, kind="stable")
        ks = key[order]
        cnt = np.bincount(key, minlength=nwin2 * NSEG)
        starts = np.concatenate([[0], np.cumsum(cnt)[:-1]])
        rank = np.arange(len(order)) - starts[ks]
        tau = tile_base[ks] + rank // P
        lane = rank % P
        p2win = np.full((P, T2), float(W2), dtype=np.float32)
        p2win[lane, tau] = slot[order]
        p2win = p2win.astype(bf16)
        idx16 = np.zeros((16, T2 * 8), dtype=np.int16)
        idx16[lane % 16, tau * 8 + lane // 16] = lidx[order].astype(np.int16)
        p2idx = np.tile(idx16, (8, 1))

        ins_maps.append(
            {
                "xs": np.ascontiguousarray(x[SHARD * k : SHARD * (k + 1)]),
                "p1win": p1win,
                "p2win": p2win,
                "p2idx": p2idx,
                "w64": w64,
                "iota1": iota1,
                "iota2": iota2,
                "ones": ones,
                "ones_m": ones_m,
                "ident": ident,
            }
        )

    meta = {
        "T1": T1,
        "T2": T2,
        "t1_w": t1_w,
        "nwin1": nwin1,
        "nwin2": nwin2,
        "sc_list": sc_list,
        "win_first": win_first,
        "win_last": win_last,
    }
    return ins_maps, meta


def _tile_maps(meta):
    # phase-1: pack whole windows into chunks of <= CHUNK1 tiles.
    chunks1 = []
    cur = []
    t0 = 0
    pos = 0
    for w, n in enumerate(meta["t1_w"]):
        n = int(n)
        if n == 0:
            continue
        if pos + n > CHUNK1 and cur:
            chunks1.append((t0, pos, cur))
            t0 += pos
            pos = 0
            cur = []
        cur.append((w, pos, pos + n))
        pos += n
    if cur:
        chunks1.append((t0, pos, cur))
    meta["p1_chunks"] = chunks1
    last_win_of_blk = {}
    for w, n in enumerate(meta["t1_w"]):
        if int(n) > 0:
            last_win_of_blk[w // 4] = w
    meta["p1_last_win_of_blk"] = last_win_of_blk
    return meta


def _build_nc(meta, do_split_waits=True, dbg=False):
    T1, T2 = meta["T1"], meta["T2"]
    t1_w = meta["t1_w"]
    nwin2 = meta["nwin2"]
    sc_list = meta["sc_list"]
    win_first = meta["win_first"]
    win_last = meta["win_last"]
    nt_max = max(sc["nt"] for sc in sc_list)

    nc = bass.Bass(num_swdge_queues=NSWQ)
    xs = nc.declare_dram_parameter("xs", [SHARD, D], F32, isOutput=False)
    p1win_d = nc.declare_dram_parameter("p1win", [P, T1], MD, isOutput=False)
    p2win_d = nc.declare_dram_parameter("p2win", [P, T2], MD, isOutput=False)
    p2idx_d = nc.declare_dram_parameter("p2idx", [P, T2 * 8], I16, isOutput=False)
    w64_d = nc.declare_dram_parameter("w64", [D, D], MD, isOutput=False)
    iota1_d = nc.declare_dram_parameter("iota1", [P, W1], MD, isOutput=False)
    iota2_d = nc.declare_dram_parameter("iota2", [P, W2], MD, isOutput=False)
    ones_d = nc.declare_dram_parameter("ones", [P, 1], F32, isOutput=False)
    onesm_d = nc.declare_dram_parameter("ones_m", [P, 1], MD, isOutput=False)
    ident_d = nc.declare_dram_parameter("ident", [D + 1, D + 1], MD, isOutput=False)
    out_d = nc.declare_dram_parameter("out", [SHARD, D], F32, isOutput=True)

    feat_s = nc.dram_tensor("feat_s", [SHARD, ELEM], MD)
    feat_fp = [
        nc.dram_tensor(f"feat_f{p}", [N_CORES * PSZ[p], ELEM], MD)
        for p in range(NSEG)
    ]
    if dbg:
        nt0 = sc_list[0]["nt"]
        dbg_gd_d = nc.declare_dram_parameter("dbg_gd", [P, nt0, ELEM], MD, isOutput=True)
        dbg_oh_d = nc.declare_dram_parameter("dbg_oh", [P, nt0, W2], MD, isOutput=True)

    with tile.TileContext(nc) as tc:
        with tc.tile_pool(name="consts", bufs=1) as consts:
            nc.gpsimd.load_library(_mlp_lib)
            w64_sb = consts.tile([D, D], MD, tag="w64")
            iota1_sb = consts.tile([P, W1], MD, tag="iota1")
            iota2_sb = consts.tile([P, W2], MD, tag="iota2")
            ones_sb = consts.tile([P, 1], F32, tag="ones")
            onesm_sb = consts.tile([P, 1], MD, tag="onesm")
            ident_sb = consts.tile([D + 1, D + 1], MD, tag="ident")
            nc.sync.dma_start(out=w64_sb[:], in_=w64_d[:])
            nc.sync.dma_start(out=iota1_sb[:], in_=iota1_d[:])
            nc.sync.dma_start(out=iota2_sb[:], in_=iota2_d[:])
            nc.sync.dma_start(out=ones_sb[:], in_=ones_d[:])
            nc.sync.dma_start(out=onesm_sb[:], in_=onesm_d[:])
            nc.sync.dma_start(out=ident_sb[:], in_=ident_d[:])
            ccsem = nc.alloc_semaphore("ccsem")

            # ---------------- phase 1: out-degree -> feat shard -------------
            with (
                tc.tile_pool(name="p1win", bufs=2) as p_win,
                tc.tile_pool(name="p1oh", bufs=2) as p_oh,
                tc.tile_pool(name="p1s", bufs=4) as p_s,
                tc.tile_pool(name="p1ps", bufs=2, space="PSUM") as p_ps,
                tc.tile_pool(name="p1trps", bufs=2, space="PSUM") as p_trps,
                tc.tile_pool(name="p1x", bufs=2) as p_x,
                tc.tile_pool(name="p1feat", bufs=2) as p_feat,
                tc.tile_pool(name="p1misc", bufs=4) as p_misc,
            ):
                ps_blk = {}

                def p1_block_epilogue(b, ps):
                    for j2 in range(4):
                        w2 = 4 * b + j2
                        if w2 >= meta["nwin1"] or t1_w[w2] == 0:
                            nc.vector.memset(ps[:, W1 * j2 : W1 * (j2 + 1)], 0.0)
                    # raw degree row [1,128] -> SBUF -> PE transpose -> [128,1]
                    rowc = p_misc.tile([1, P], F32, tag="m_row")
                    nc.vector.tensor_copy(rowc[:], ps[:])
                    tp = p_trps.tile([P, 1], F32)
                    nc.tensor.matmul(
                        out=tp[:],
                        lhsT=rowc[:],
                        rhs=ones_sb[0:1, 0:1],
                        start=True,
                        stop=True,
                    )
                    dcl = p_misc.tile([P, 1], F32, tag="m_dcl")
                    nc.vector.tensor_scalar_max(dcl[:], tp[:], 1.0)
                    dsq = p_misc.tile([P, 1], F32, tag="m_dsq")
                    nc.scalar.sqrt(dsq[:], dcl[:])
                    ncol = p_misc.tile([P, 1], F32, tag="m_ncol")
                    nc.vector.reciprocal(ncol[:], dsq[:])
                    nb = min(P, SHARD - P * b)
                    xb = p_x.tile([P, D], F32, tag="xb")
                    nc.sync.dma_start(out=xb[:nb], in_=xs[P * b : P * b + nb, :])
                    fb = p_feat.tile([P, ELEM], MD, tag="fb")
                    nc.scalar.mul(fb[:, 0:D], xb[:], ncol[:])
                    nc.vector.memset(fb[:, D : D + 1], 1.0)
                    nc.vector.memset(fb[:, D + 1 : ELEM], 0.0)
                    nc.sync.dma_start(
                        out=feat_s[P * b : P * b + nb, :], in_=fb[:nb, :]
                    )

                # AllGather piece p covers shard rows [PSTART[p], PSTART[p]+PSZ[p]);
                # issued as soon as its last 128-row block is written, overlapping
                # the collective with the rest of phase 1.
                piece_end_blk = {
                    (PSTART[p] + PSZ[p] + P - 1) // P - 1: p for p in range(NSEG)
                }

                def emit_allgather(p):
                    with tc.tile_critical():
                        nc.gpsimd.collective_compute(
                            "AllGather",
                            mybir.AluOpType.bypass,
                            replica_groups=[list(range(N_CORES))],
                            ins=[feat_s[PSTART[p] : PSTART[p] + PSZ[p], :]],
                            outs=[feat_fp[p][:]],
                        ).then_inc(ccsem, 1)

                def maybe_allgather(b):
                    p = piece_end_blk.get(b)
                    if p is not None:
                        emit_allgather(p)

                for t0, cw, wins in meta["p1_chunks"]:
                    wt = p_win.tile([P, CHUNK1], MD, tag="wt")
                    nc.sync.dma_start(out=wt[:, :cw], in_=p1win_d[:, t0 : t0 + cw])
                    oh = p_oh.tile([P, W1, CHUNK1], MD, tag="oh")
                    nc.vector.tensor_tensor(
                        out=oh[:, :, :cw],
                        in0=wt[:, None, :cw].to_broadcast([P, W1, cw]),
                        in1=iota1_sb[:, :, None].to_broadcast([P, W1, cw]),
                        op=mybir.AluOpType.is_equal,
                    )
                    for w, a, bnd in wins:
                        S = p_s.tile([P, W1, 1], MD, tag="S")
                        with nc.allow_low_precision(
                            reason="one-hot counts <=64 are exact in bf16"
                        ):
                            nc.vector.tensor_reduce(
                                out=S[:],
                                in_=oh[:, :, a:bnd],
                                axis=mybir.AxisListType.X,
                                op=mybir.AluOpType.add,
                            )
                        b, j = w // 4, w % 4
                        if b not in ps_blk:
                            ps_blk[b] = p_ps.tile([1, P], F32, name="psblk", tag="psblk")
                        nc.tensor.matmul(
                            out=ps_blk[b][:, W1 * j : W1 * (j + 1)],
                            lhsT=onesm_sb[:],
                            rhs=S[:, :, 0],
                            start=True,
                            stop=True,
                        )
                        if w == meta["p1_last_win_of_blk"].get(b, -1):
                            p1_block_epilogue(b, ps_blk.pop(b))
                            maybe_allgather(b)

            # -------- phase 2: batched gather + scatter matmul + W ----------
            with (
                tc.tile_pool(name="p2i", bufs=3) as p_idx,
                tc.tile_pool(name="p2w", bufs=3) as p_win2,
                tc.tile_pool(name="p2g", bufs=3) as p_g,
                tc.tile_pool(name="p2oh", bufs=3) as p_oh2,
                tc.tile_pool(name="p2ps", bufs=3, space="PSUM") as p_ps2,
                tc.tile_pool(name="p2tr", bufs=3, space="PSUM") as p_tr2,
                tc.tile_pool(name="p2ops", bufs=2, space="PSUM") as p_ops,
                tc.tile_pool(name="p2mrg", bufs=4) as p_mrg,
                tc.tile_pool(name="p2out", bufs=4) as p_out,
                tc.tile_pool(name="p2misc", bufs=8) as p_misc2,
            ):
                nreg = {}
                for sc in sc_list:
                    for q, lt0, n in sc["calls"]:
                        if n * P not in nreg:
                            nreg[n * P] = nc.gpsimd.to_reg(n * P)
                gcall_i = 0
                seg_waited = set()
                for sc in sc_list:
                    nt = sc["nt"]
                    if nt == 0:
                        continue
                    t0 = sc["t0"]
                    ix = p_idx.tile([P, nt_max * 8], I16, tag="ix")
                    nc.sync.dma_start(
                        out=ix[:, : nt * 8], in_=p2idx_d[:, t0 * 8 : (t0 + nt) * 8]
                    )
                    wt = p_win2.tile([P, nt_max], MD, tag="wt2")
                    nc.sync.dma_start(out=wt[:, :nt], in_=p2win_d[:, t0 : t0 + nt])
                    gd = p_g.tile([P, nt_max, ELEM], MD, tag="gd")
                    oh = p_oh2.tile([P, nt_max, W2], MD, tag="oh2")
                    for q, lt0, n in sc["calls"]:
                        if q not in seg_waited:
                            with tc.tile_critical():
                                nc.gpsimd.wait_ge(ccsem, q + 1)
                            seg_waited.add(q)
                        nc.gpsimd.dma_gather(
                            gd[:, lt0 : lt0 + n, :],
                            feat_fp[q][:],
                            ix[:, lt0 * 8 : (lt0 + n) * 8],
                            n * P,
                            nreg[n * P],
                            ELEM,
                            queue_num=gcall_i % NSWQ,
                        )
                        gcall_i += 1
                        nc.vector.tensor_tensor(
                            out=oh[:, lt0 : lt0 + n, :],
                            in0=wt[:, lt0 : lt0 + n, None].to_broadcast([P, n, W2]),
                            in1=iota2_sb[:, None, :].to_broadcast([P, n, W2]),
                            op=mybir.AluOpType.is_equal,
                        )
                    for w in sc["ws"]:
                        lts = sc["wtiles"][w]
                        if not lts:
                            continue
                        ps = p_ps2.tile([D + 1, W2], F32, tag="ps2")
                        for i, lt in enumerate(lts):
                            nc.tensor.matmul(
                                out=ps[:],
                                lhsT=gd[:, lt, 0 : D + 1],
                                rhs=oh[:, lt, :],
                                start=(i == 0),
                                stop=(i == len(lts) - 1),
                            )
                        # norm_dst from the exact deg row, via transpose
                        rowc = p_misc2.tile([1, P], F32, tag="d_row")
                        nc.vector.tensor_copy(rowc[:], ps[D : D + 1, :])
                        tp2 = p_tr2.tile([P, 1], F32)
                        nc.tensor.matmul(
                            out=tp2[:],
                            lhsT=rowc[:],
                            rhs=ones_sb[0:1, 0:1],
                            start=True,
                            stop=True,
                        )
                        dcl = p_misc2.tile([P, 1], F32, tag="d_dcl")
                        nc.vector.tensor_scalar_max(dcl[:], tp2[:], 1.0)
                        dsq = p_misc2.tile([P, 1], F32, tag="d_dsq")
                        nc.scalar.sqrt(dsq[:], dcl[:])
                        drr = p_misc2.tile([P, 1], F32, tag="d_drr")
                        nc.vector.reciprocal(drr[:], dsq[:])
                        ag = p_mrg.tile([D, P], MD, tag="agf")
                        nc.scalar.copy(ag[:], ps[0:D, :])
                        op = p_ops.tile([P, D], F32)
                        nc.tensor.matmul(
                            out=op[:],
                            lhsT=ag[:],
                            rhs=w64_sb[:],
                            start=True,
                            stop=True,
                        )
                        ob = p_out.tile([P, D], F32, tag="ob")
                        nc.scalar.mul(ob[:], op[:], drr[:])
                        nb = min(P, SHARD - W2 * w)
                        nc.sync.dma_start(
                            out=out_d[W2 * w : W2 * w + nb, :], in_=ob[:nb, :]
                        )
                # windows with no edges anywhere: write zeros
                for w in range(nwin2):
                    if w not in win_first:
                        zb = p_out.tile([P, D], F32, tag="ob")
                        nc.vector.memset(zb[:], 0.0)
                        nb = min(P, SHARD - W2 * w)
                        nc.sync.dma_start(
                            out=out_d[W2 * w : W2 * w + nb, :], in_=zb[:nb, :]
                        )

    if do_split_waits:
        split_waits(nc)
    hoist_library_reload(nc)
    mybir.codegen_inst_isa_subclasses(nc)
    return nc


def kernel(x, W, src, dst):
    from concourse.bass_utils import run_bass_kernel_spmd

    ins_maps, meta = _prep(x, W, src, dst)
    meta = _tile_maps(meta)
    nc = _build_nc(meta)
    res = run_bass_kernel_spmd(nc, ins_maps, list(range(N_CORES)))
    out = np.concatenate([res.results[k]["out"] for k in range(N_CORES)], axis=0)
    return out.astype(np.float32)
